# revision 1
# baseline (speedup 1.0000x reference)
"""Trainium2 Bass kernel for a 2-state linear-chain CRF loss (BiLSTM-CRF loss_fn).

Computes, for a single conversation of length T = 2,097,152:
  gold_score  = sum_t em[t, lab[t]] + sum_{t>0} trans[t][lab[t-1], lab[t]]
  total_score = logsumexp of the CRF forward recursion
where trans[t] = who2who_sub[w[t]] + position_sub[p[t]] (60 possible 2x2
matrices; indices 2/19 select an all-zero padding matrix).

Design (one NeuronCore per contiguous chunk of 262,144 steps, 8 cores):

* Forward pass: the recursion is a product of 2x2 matrices in the (log, +)
  semiring, which is associative, so each core tree-reduces its chunk
  (11 in-partition levels + a 7-level tail across partitions) with
  LSE(a, b) = a + softplus(b - a), softplus composed as Ln(exp(d) + 1) on
  the ACT engine (both functions live in one ACT table set; the alternating
  per-function table reloads bacc would emit are deduplicated post-compile).
  The host multiplies the 8 chunk matrices in order (7 tiny 2x2 products).

* Per-step matrices: trans is built by per-class masked accumulation
  (19 position + 2 who2who classes).  Each (class, component) is one fused
  fp16 tensor_scalar mv = (idx == c) * V_c (fast 2-byte DVE mode, triple-
  buffered per component) plus one fp16 add; the adds are exact because
  class masks are disjoint.  The serial add chains are split across
  engines: GPSIMD (slower per add but otherwise idle) takes all of
  component 3 plus the tails of components 2 and 1, tuned so all four
  chains finish together just before the tree consumes them.

* Gold score: fp16 tables would bias the selected-transition sum (each
  entry's rounding repeats identically in every chunk), so gold instead
  counts occurrences of each (class, label-pair) cell -- integer counts,
  exact in f32 -- and dots the counts with the full-precision f32
  parameter row.  Cells below NACT_LO are one fused DVE
  tensor_scalar(is_equal, accum_out=add) each; the rest run on the ACT
  engine as Relu(1 - (idx - cell)^2) masks with accum_out (exact for
  small-integer indices).  The emission part of gold runs on GPSIMD; fp16
  emissions cost only ~1e-6 relative on gold.

* All inputs ship as a single per-core fp16 blob
  [p | par(f32 bytes) | w | lab | labp | em] (3.1 MB/core); the p+par head
  is its own small DMA so the class-sum starts immediately.  Packing one
  blob keeps every instruction dependent on at most one DMA semaphore
  (trn2 instructions carry a single sync-wait slot; bacc's
  generate_event_semaphores legalizes any remainder).

The host only reshapes/casts/shards inputs and combines the 8 per-core
results; all O(T) work happens on-device.  Accuracy vs the fp32 jax
reference: gold ~2e-6 rel; total ~1e-3 rel, which is the reference's own
sequential-fp32-scan rounding wander at T=2M (a float64 ground truth sits
on our side of it).
"""

from contextlib import ExitStack

import numpy as np

import concourse.bass as bass
import concourse.bacc as bacc
import concourse.mybir as mybir
import concourse.tile as tile
from concourse import bass_utils

dt = mybir.dt
ALU = mybir.AluOpType
AF = mybir.ActivationFunctionType
AX = mybir.AxisListType

T = 2097152
NCORES = 8
P = 128                  # SBUF partitions
L = T // NCORES          # steps per core = 262144
F = L // P               # steps per partition = 2048
H = F // 2               # pairs per partition at level 1
NPOS = 19                # position classes with nonzero matrices (19 -> zero pad)
NPAR = 4 * NPOS + 8      # packed param row: 19 position + 2 who2who matrices
E = 5                    # packed result width: 4 matrix entries + gold partial
NACT_LO = 62             # count cells >= this id run on the ACT engine
W0 = 6 * F + 2 * NPAR    # blob0 (fp16): [p | par | w | lab | labp | em]


_NC_CACHE = None
LAST_RESULTS = None  # BassKernelResults of the most recent kernel() call


def _comp(i, j):
    return i * 2 + j


def _build_nc():
    nc = bacc.Bacc()

    b0_d = nc.dram_tensor("blob0", [P, W0], dt.float16, kind="ExternalInput")
    out_d = nc.dram_tensor("out", [1, 8], dt.float32, kind="ExternalOutput")

    # const APs for the ACT-side count masks: Square bias=-cell, Relu scale=-1
    for _v in sorted({-float(c) for c in range(NACT_LO, 4 * NPOS + 8)} | {-1.0}):
        if (dt.float32, _v) in nc.const_aps.aps:
            continue
        _t = nc.alloc_sbuf_tensor(f"const-float32-{_v}", [128, 1], dt.float32)
        nc.gpsimd.memset(_t.ap(), _v)
        nc.const_aps.aps[(dt.float32, _v)] = _t.ap()
    nc.all_engine_barrier()

    with ExitStack() as ctx:
        tc = ctx.enter_context(tile.TileContext(nc))
        pool = ctx.enter_context(tc.tile_pool(name="main", bufs=1))
        dpool = ctx.enter_context(tc.tile_pool(name="dram", bufs=1, space="DRAM"))

        # ---- loads ----
        # blob0 layout: [p | par | w | lab | labp]; the p+par head ships as
        # its own small DMA so the class-sum starts ~10us earlier.
        b0 = pool.tile([P, W0], dt.float16, tag="b0", name="b0")
        head = F + 2 * NPAR
        nc.sync.dma_start(b0[:, 0:head], b0_d[:, 0:head])
        nc.sync.dma_start(b0[:, head:W0], b0_d[:, head:W0])

        p_t = b0[:, 0:F]
        par32 = b0[:, F:head].bitcast(dt.float32)
        w_t = b0[:, head:head + F]
        lab16 = b0[:, head + F:head + 2 * F]
        labp16 = b0[:, head + 2 * F:head + 3 * F]
        em_t = b0[:, head + 3 * F:head + 5 * F].rearrange("p (f c) -> p f c", c=2)

        def V(col):
            return par32[:, col:col + 1]

        # ---- per-class masked accumulation of trans components ----
        # mv = (idx == c) * V_c in one fused fp16 tensor_scalar (fast 2-byte
        # mode); the accumulate adds are exact because class masks are
        # disjoint (acc only ever gains one nonzero term per table).
        acc = [
            pool.tile([P, F], dt.float16, tag=f"acc{c}", name=f"acc{c}")
            for c in range(4)
        ]
        # two mv buffers per component: (i*4+comp) % 4 would alias to one
        # buffer per comp, stalling the DVE producer at the GP consumer's pace
        mv = [
            pool.tile([P, F], dt.float16, tag=f"mv{i}", name=f"mv{i}")
            for i in range(12)
        ]
        for comp in range(4):
            nc.vector.tensor_scalar(
                acc[comp][:], p_t, 0.0, V(comp), ALU.is_equal, ALU.mult
            )
        classes = [(p_t, float(c), 4 * c) for c in range(1, NPOS)]
        classes += [(w_t, float(v), 4 * NPOS + 4 * v) for v in range(2)]
        for i, (src, cval, col) in enumerate(classes):
            for comp in range(4):
                m = mv[(i % 3) * 4 + comp]
                nc.vector.tensor_scalar(
                    m[:], src, cval, V(col + comp), ALU.is_equal, ALU.mult
                )
                # The serial accumulate chains are split across engines:
                # GPSIMD (3.4x slower per add but otherwise idle) takes all
                # of comp 3 plus the tails of comps 2 and 1, so the four
                # chains finish together just before the tree needs them
                # (split points tuned against the instruction cost model).
                on_gp = (comp == 3 or (comp == 2 and i >= 15)
                         or (comp == 1 and i >= 18))
                eng = nc.gpsimd if on_gp else nc.vector
                eng.tensor_add(acc[comp][:], acc[comp][:], m[:])

        # ---- gold score ----
        # The fp16 acc would bias the selected-transition sum (the fp16
        # rounding of each table entry repeats identically in every chunk),
        # so gold instead counts how often each (class, label-pair) cell
        # occurs -- integer counts, exact in f32 -- and dots the counts with
        # the full-precision f32 param row.  One fused fp16 tensor_scalar
        # (is_equal + accum_out) per cell.
        msel = pool.tile([P, F], dt.float16, tag="msel", name="msel")
        nc.vector.tensor_scalar(msel[:], labp16, 2.0, None, ALU.mult)
        nc.vector.tensor_add(msel[:], msel[:], lab16)
        # joint indices: 4*p + lpc and 4*w + lpc (exact small ints in fp16)
        jp = pool.tile([P, F], dt.float16, tag="jp", name="jp")
        nc.vector.tensor_scalar(jp[:], p_t, 4.0, None, ALU.mult)
        nc.vector.tensor_add(jp[:], jp[:], msel[:])
        jw = pool.tile([P, F], dt.float16, tag="jw", name="jw")
        nc.vector.tensor_scalar(jw[:], w_t, 4.0, None, ALU.mult)
        nc.vector.tensor_add(jw[:], jw[:], msel[:])
        cnt = pool.tile([P, NPAR], dt.float32, tag="cnt", name="cnt")
        junk = [
            pool.tile([P, F], dt.float16, tag=f"junk{i}", name=f"junk{i}")
            for i in range(2)
        ]
        ajunk = [
            pool.tile([P, F], dt.float16, tag=f"ajunk{i}", name=f"ajunk{i}")
            for i in range(2)
        ]

        def count_cell(src, cellv, col):
            if cellv >= NACT_LO:
                # ACT-side: mask = Relu(1 - (idx - cell)^2), sum via accum_out
                a = ajunk[col % 2]
                nc.scalar.activation(a[:], src, AF.Square, bias=-float(cellv))
                nc.scalar.activation(
                    a[:], a[:], AF.Relu, bias=1.0, scale=-1.0,
                    accum_out=cnt[:, col:col + 1],
                )
            else:
                nc.vector.tensor_scalar(
                    junk[col % 2][:], src, float(cellv), None, ALU.is_equal,
                    ALU.add, accum_out=cnt[:, col:col + 1],
                )

        for cell in range(4 * NPOS):
            count_cell(jp[:], cell, cell)
        for cell in range(8):
            count_cell(jw[:], cell, 4 * NPOS + cell)
        cntv = pool.tile([P, NPAR], dt.float32, tag="cntv", name="cntv")
        nc.vector.tensor_mul(cntv[:], cnt[:], par32[:, 0:NPAR])
        gold_tr = pool.tile([P, 1], dt.float32, tag="gold_tr", name="gold_tr")
        nc.vector.reduce_sum(gold_tr[:], cntv[:], axis=AX.X)
        # emission part stays exact f32
        em0 = em_t[:, :, 0]
        em1 = em_t[:, :, 1]
        demm = pool.tile([P, F], dt.float16, tag="demm", name="demm")
        nc.gpsimd.tensor_sub(demm[:], em1, em0)
        nc.gpsimd.tensor_mul(demm[:], demm[:], lab16)
        nc.gpsimd.tensor_add(demm[:], demm[:], em0)
        gold_part = pool.tile([P, 1], dt.float32, tag="gold_part", name="gold_part")
        nc.vector.reduce_sum(gold_part[:], demm[:], axis=AX.X)
        nc.vector.tensor_add(gold_part[:], gold_part[:], gold_tr[:])

        # ---- fold emissions into trans: M[i,j] = trans[i,j] + em[j] ----
        for i in range(2):
            for j in range(2):
                a = acc[_comp(i, j)]
                eng = nc.gpsimd if _comp(i, j) >= 2 else nc.vector
                eng.tensor_add(a[:], a[:], em_t[:, :, j])

        # ---- level 1: combine adjacent step pairs from the separated tiles ----
        # Levels 1-4 run their adds in fp16 (values <= ~25, 2x DVE rate; the
        # softplus intermediate stays f32 -- fp16 exp would overflow past
        # d ~ 11).  Levels 5+ use the original f32 in-place flow.
        FP16_LEVELS = 4
        X16 = pool.tile([P, H, 2, 2], dt.float16, tag="X16", name="X16")
        Y16a = pool.tile([P, H, 2, 2], dt.float16, tag="Y16a", name="Y16a")
        Y16b = pool.tile([P, H // 2, 2, 2], dt.float16, tag="Y16b", name="Y16b")
        X32 = pool.tile([P, H // 16, 2, 2], dt.float32, tag="X32", name="X32")
        # ping-pong softplus buffers: the ACT engine can carry only one
        # sync-wait, so its WAW target must be >=2 ACT-instructions old
        Y0 = pool.tile([P, H, 2, 2], dt.float32, tag="Y0", name="Y0")
        Y1 = pool.tile([P, H // 2, 2, 2], dt.float32, tag="Y1", name="Y1")

        def u2(ap):
            return ap.unsqueeze(2).unsqueeze(3)

        for i in range(2):
            for j in range(2):
                # x[i,j] = A[i,0] + B[0,j];  y[i,j] = A[i,1] + B[1,j]
                nc.vector.tensor_add(
                    X16[:, :, i:i + 1, j:j + 1],
                    u2(acc[_comp(i, 0)][:, 0::2]),
                    u2(acc[_comp(0, j)][:, 1::2]),
                )
                nc.vector.tensor_add(
                    Y16a[:, :, i:i + 1, j:j + 1],
                    u2(acc[_comp(i, 1)][:, 0::2]),
                    u2(acc[_comp(1, j)][:, 1::2]),
                )
        nc.vector.tensor_sub(Y16a[:], Y16a[:], X16[:])
        nc.scalar.activation(Y0[:], Y16a[:], AF.Exp)
        nc.scalar.activation(Y0[:], Y0[:], AF.Ln, bias=1.0)
        mlev = pool.tile([P, H, 2, 2], dt.float16, tag="m1", name="m1")
        nc.vector.tensor_add(mlev[:], X16[:], Y0[:])

        # ---- levels 2..11: interleaved tree reduction along the free dim ----
        w_cur = H
        lev = 1
        while w_cur > 1:
            w2 = w_cur // 2
            lev += 1
            sh = [P, w2, 2, 2]
            a_i0 = mlev[:, 0:w_cur:2, :, 0:1].broadcast_to(sh)
            a_i1 = mlev[:, 0:w_cur:2, :, 1:2].broadcast_to(sh)
            b_0j = mlev[:, 1:w_cur:2, 0:1, :].broadcast_to(sh)
            b_1j = mlev[:, 1:w_cur:2, 1:2, :].broadcast_to(sh)
            sp = (Y0 if lev % 2 == 1 else Y1)[:, 0:w2]
            if lev <= FP16_LEVELS:
                xv = X16[:, 0:w2]
                yv = (Y16a if lev % 2 == 1 else Y16b)[:, 0:w2]
                nc.vector.tensor_add(xv, a_i0, b_0j)
                nc.vector.tensor_add(yv, a_i1, b_1j)
                nc.vector.tensor_sub(yv, yv, xv)
                nc.scalar.activation(sp, yv, AF.Exp)
            else:
                xv = X32[:, 0:w2]
                yv = sp
                nc.vector.tensor_add(xv, a_i0, b_0j)
                nc.vector.tensor_add(yv, a_i1, b_1j)
                nc.vector.tensor_sub(yv, yv, xv)
                nc.scalar.activation(sp, sp, AF.Exp)
            nc.scalar.activation(sp, sp, AF.Ln, bias=1.0)
            mdt = dt.float16 if lev <= FP16_LEVELS else dt.float32
            mnext = pool.tile(sh, mdt, tag=f"m{lev}", name=f"m{lev}")
            nc.vector.tensor_add(mnext[:], xv, sp)
            mlev = mnext
            w_cur = w2

        # ---- pack per-partition results and bounce through DRAM to one row ----
        pk = pool.tile([P, E], dt.float32, tag="pk", name="pk")
        nc.vector.tensor_copy(
            pk[:, 0:4].rearrange("p (a b c) -> p a b c", a=1, b=2), mlev[:]
        )
        nc.vector.tensor_copy(pk[:, 4:5], gold_part[:])
        scr = dpool.tile([P, E], dt.float32, tag="scr", name="scr")
        nc.sync.dma_start(scr[:], pk[:])
        fin = pool.tile([1, P * E], dt.float32, tag="fin", name="fin")
        nc.sync.dma_start(fin[:], scr[:].rearrange("p e -> (p e)").unsqueeze(0))
        v = fin[:].rearrange("o (p e) -> o p e", e=E)

        gold_tot = pool.tile([1, 1], dt.float32, tag="gold_tot", name="gold_tot")
        nc.vector.reduce_sum(gold_tot[:], v[:, :, 4], axis=AX.X)

        # ---- tail tree over the 128 per-partition chunk matrices ----
        TX = pool.tile([1, P // 2, 2, 2], dt.float32, tag="TX", name="TX")
        TY0 = pool.tile([1, P // 2, 2, 2], dt.float32, tag="TY0", name="TY0")
        TY1 = pool.tile([1, P // 2, 2, 2], dt.float32, tag="TY1", name="TY1")
        w2 = P // 2
        sh = [1, w2, 2, 2]
        a_i0 = v[:, 0::2, 0:3:2].unsqueeze(3).broadcast_to(sh)
        a_i1 = v[:, 0::2, 1:4:2].unsqueeze(3).broadcast_to(sh)
        b_0j = v[:, 1::2, 0:2].unsqueeze(2).broadcast_to(sh)
        b_1j = v[:, 1::2, 2:4].unsqueeze(2).broadcast_to(sh)
        xv = TX[:, 0:w2]
        yv = TY0[:, 0:w2]
        nc.vector.tensor_add(xv, a_i0, b_0j)
        nc.vector.tensor_add(yv, a_i1, b_1j)
        nc.vector.tensor_sub(yv, yv, xv)
        nc.scalar.activation(yv, yv, AF.Exp)
        nc.scalar.activation(yv, yv, AF.Ln, bias=1.0)
        tlev = pool.tile(sh, dt.float32, tag="t1", name="t1")
        nc.vector.tensor_add(tlev[:], xv, yv)
        w_cur = w2
        lev = 1
        while w_cur > 1:
            w2 = w_cur // 2
            lev += 1
            sh = [1, w2, 2, 2]
            a_i0 = tlev[:, 0:w_cur:2, :, 0:1].broadcast_to(sh)
            a_i1 = tlev[:, 0:w_cur:2, :, 1:2].broadcast_to(sh)
            b_0j = tlev[:, 1:w_cur:2, 0:1, :].broadcast_to(sh)
            b_1j = tlev[:, 1:w_cur:2, 1:2, :].broadcast_to(sh)
            xv = TX[:, 0:w2]
            yv = (TY0 if lev % 2 == 1 else TY1)[:, 0:w2]
            nc.vector.tensor_add(xv, a_i0, b_0j)
            nc.vector.tensor_add(yv, a_i1, b_1j)
            nc.vector.tensor_sub(yv, yv, xv)
            nc.scalar.activation(yv, yv, AF.Exp)
            nc.scalar.activation(yv, yv, AF.Ln, bias=1.0)
            tnext = pool.tile(sh, dt.float32, tag=f"t{lev}", name=f"t{lev}")
            nc.vector.tensor_add(tnext[:], xv, yv)
            tlev = tnext
            w_cur = w2

        # ---- assemble [P00, P01, P10, P11, gold, 0, 0, 0] and store ----
        res = pool.tile([1, 8], dt.float32, tag="res", name="res")
        nc.vector.memset(res[:], 0.0)
        nc.vector.tensor_copy(
            res[:, 0:4].rearrange("p (a b c) -> p a b c", a=1, b=2), tlev[:]
        )
        nc.vector.tensor_copy(res[:, 4:5], gold_tot[:])
        nc.sync.dma_start(out_d[:], res[:])

    nc.compile()

    # Both Exp and Ln live in the 'natural_log_exp_and_others' ACT table set,
    # but insert_act_table_loads picks the first set containing each function,
    # emitting an alternating exp/ln reload (1.3 us each) per tree level.
    # Retarget every load to the combined set and drop the now-redundant ones
    # (none carry sync_info).
    from concourse.hw_specs import get_activation_tables

    tables = list(get_activation_tables(nc.m.arch).keys())
    combined = tables.index("natural_log_exp_and_others")
    for b in nc.bb_map.values():
        insts = b.bb.instructions
        kept = []
        seen_load = False
        for ins in insts:
            if ins.opcode == "LoadActFuncSet":
                si = ins.sync_info
                assert not (si and (si.on_wait or si.on_update)), ins.name
                if seen_load:
                    continue
                ins.act_func_set_id = combined
                seen_load = True
            kept.append(ins)
        if len(kept) != len(insts):
            b.bb.instructions = kept
    return nc


def _get_nc():
    global _NC_CACHE
    if _NC_CACHE is None:
        _NC_CACHE = _build_nc()
    return _NC_CACHE


def kernel(**inputs):
    em = np.asarray(inputs["emission_scores"], dtype=np.float32)
    lab = np.asarray(inputs["label"]).astype(np.float32)
    w = np.asarray(inputs["who2who_state"]).astype(np.float32)
    p = np.asarray(inputs["position_state"]).astype(np.float32)
    w2w = np.asarray(inputs["who2who_params"], dtype=np.float32)
    pos = np.asarray(inputs["position_params"], dtype=np.float32)
    assert em.shape == (T, 2), em.shape

    labp = np.empty_like(lab)
    labp[0] = 0.0
    labp[1:] = lab[:-1]

    # single fp16 blob: [p | par(f32 bytes as fp16 pairs) | w | lab | labp | em]
    par_row = np.concatenate([pos.reshape(-1), w2w.reshape(-1)]).astype(np.float32)
    par16 = np.broadcast_to(par_row.view(np.float16), (P, 2 * NPAR))
    p16 = p.astype(np.float16)
    w16 = w.astype(np.float16)
    lab16 = lab.astype(np.float16)
    labp16 = labp.astype(np.float16)
    em16 = em.astype(np.float16)

    in_maps = []
    for k in range(NCORES):
        sl = slice(k * L, (k + 1) * L)
        blob0 = np.concatenate(
            [
                p16[sl].reshape(P, F),
                par16,
                w16[sl].reshape(P, F),
                lab16[sl].reshape(P, F),
                labp16[sl].reshape(P, F),
                em16[sl].reshape(P, 2 * F),
            ],
            axis=1,
        )
        in_maps.append({"blob0": np.ascontiguousarray(blob0)})

    nc = _get_nc()
    kr = bass_utils.run_bass_kernel_spmd(nc, in_maps, core_ids=list(range(NCORES)))
    global LAST_RESULTS
    LAST_RESULTS = kr
    results = kr.results

    # host combine: 7 log-semiring 2x2 products (in order) + gold partial sum
    mats = []
    gold = 0.0
    for r in results:
        row = np.asarray(r["out"], dtype=np.float64).reshape(-1)
        mats.append(row[0:4].reshape(2, 2))
        gold += row[4]
    U = mats[0]
    for M in mats[1:]:
        U = np.logaddexp(U[:, 0:1] + M[0:1, :], U[:, 1:2] + M[1:2, :])
    total = np.logaddexp.reduce(U.reshape(-1))
    return np.stack([gold, total]).astype(np.float32)


if __name__ == "__main__":
    rng = np.random.default_rng(0)
    demo = dict(
        emission_scores=rng.standard_normal((T, 2)).astype(np.float32),
        label=rng.integers(0, 2, T),
        who2who_state=np.concatenate([[2], rng.integers(0, 2, T - 1)]),
        position_state=np.concatenate([[19], rng.integers(0, 19, T - 1)]),
        who2who_params=rng.standard_normal((2, 2, 2)).astype(np.float32),
        position_params=rng.standard_normal((19, 2, 2)).astype(np.float32),
    )
    print(kernel(**demo))



# revision 3
# speedup vs baseline: 1.0973x; 1.0973x over previous
"""Trainium2 Bass kernel for a 2-state linear-chain CRF loss (BiLSTM-CRF loss_fn).

Computes, for a single conversation of length T = 2,097,152:
  gold_score  = sum_t em[t, lab[t]] + sum_{t>0} trans[t][lab[t-1], lab[t]]
  total_score = logsumexp of the CRF forward recursion
where trans[t] = who2who_sub[w[t]] + position_sub[p[t]] (60 possible 2x2
matrices; indices 2/19 select an all-zero padding matrix).

Design (one NeuronCore per contiguous chunk of 262,144 steps, 8 cores):

* Per-step matrices: trans+em is built as 4 fp16 streams by per-class masked
  accumulation (19 position classes + 2 who2who classes + emission fold).
  Class masks are disjoint, so every accumulate add is exact.  The work is
  split three ways: DVE runs fused (idx==c)*V tensor_scalars (4x fp16 mode)
  plus adds; the ACT engine produces masked values for a suffix of position
  classes as Relu((V+B) - (V+B)*(p-c)^2) with B=4 making the peak positive
  (the spurious +B*[p>=a] is removed by one (p>a-.5)*B mask and 4 subtracts;
  the t=0 pad step ends shifted by exactly -B, corrected on the host); the
  GPSIMD engine takes a tuned share of the accumulate adds.

* Gold score: the label-pair stream msel = 2*lab[t-1]+lab[t] selects one of
  the 4 finished streams per step; gold = sum_t acc[msel_t][t] via 4
  is_equal masks + multiply + accum_out per sub-chunk.  Exactness: stream
  values are single-fp16-rounded table values (+ exact-in-fp16 shifts), and
  the host chooses each table entry's fp16 rounding DIRECTION (greedy sign
  optimization over the 19x2x4 reachable cells) so the systematic selection
  bias cancels to ~1e-5 relative.

* Forward pass: the recursion is a product of 2x2 matrices in the (log, +)
  semiring; each core tree-reduces with LSE(a,b) = a + ln(1+exp(b-a)) on
  ACT.  The chunk is split into SC=2 sub-chunks so sub-chunk 0's tree
  overlaps sub-chunk 1's stream build; a final in-partition level combines
  them, then a DRAM bounce + 7-level tail tree reduces the 128 partition
  matrices.  The host multiplies the 8 chunk matrices (7 tiny products).

* All inputs ship as one fp16 blob [par | p | w | msel | em0 | em1] in 3
  DMAs so the class masks start immediately.
"""

from contextlib import ExitStack

import numpy as np

import concourse.bass as bass
import concourse.bacc as bacc
import concourse.mybir as mybir
import concourse.tile as tile
from concourse import bass_utils

dt = mybir.dt
ALU = mybir.AluOpType
AF = mybir.ActivationFunctionType
AX = mybir.AxisListType

T = 2097152
NCORES = 8
P = 128                  # SBUF partitions
L = T // NCORES          # steps per core = 262144
F = L // P               # steps per partition = 2048
SC = 2                   # sub-chunks per partition row
FS = F // SC             # steps per partition per sub-chunk
NPOS = 19                # position classes with nonzero matrices
BSH = 4.0                # ACT positivity shift
ACT_LO = 13              # position classes >= this use ACT-produced mv
E = 5                    # packed result width: 4 matrix entries + gold

# param row layout (f32 words): [pos 19*4 | w2w 2*4 | VB 19*4 | negVB 19*4]
NPAR = 19 * 4 + 8 + 19 * 4 + 19 * 4
COL_POS = 0
COL_W = 76
COL_VB = 84
COL_NVB = 160

W0 = 2 * NPAR + 5 * F    # fp16 blob columns


_NC_CACHE = None
LAST_RESULTS = None  # BassKernelResults of the most recent kernel() call


def _comp(i, j):
    return i * 2 + j


def _build_nc():
    nc = bacc.Bacc()

    b0_d = nc.dram_tensor("blob0", [P, W0], dt.float16, kind="ExternalInput")
    out_d = nc.dram_tensor("out", [1, 8], dt.float32, kind="ExternalOutput")

    with ExitStack() as ctx:
        tc = ctx.enter_context(tile.TileContext(nc))
        pool = ctx.enter_context(tc.tile_pool(name="main", bufs=1))
        dpool = ctx.enter_context(tc.tile_pool(name="dram", bufs=1, space="DRAM"))

        # ---- loads: [par | p | w | msel | em0 | em1] in 3 DMAs ----
        b0 = pool.tile([P, W0], dt.float16, tag="b0", name="b0")
        h0 = 2 * NPAR + F          # par + p
        h1 = h0 + 2 * F            # + w + msel
        nc.sync.dma_start(b0[:, 0:h0], b0_d[:, 0:h0])
        nc.sync.dma_start(b0[:, h0:h1], b0_d[:, h0:h1])
        nc.sync.dma_start(b0[:, h1:W0], b0_d[:, h1:W0])

        par32 = b0[:, 0:2 * NPAR].bitcast(dt.float32)
        p_t = b0[:, 2 * NPAR:h0]
        w_t = b0[:, h0:h0 + F]
        msel_t = b0[:, h0 + F:h1]
        em0_t = b0[:, h1:h1 + F]
        em1_t = b0[:, h1 + F:W0]

        def V(col):
            return par32[:, col:col + 1]

        # small consts / biases (AP form, f32)
        bias_c = {}
        for c in range(ACT_LO, NPOS):
            t_ = pool.tile([P, 1], dt.float32, tag=f"bc{c}", name=f"bc{c}")
            nc.vector.memset(t_[:], -float(c))
            bias_c[c] = t_
        bconst = pool.tile([P, 1], dt.float32, tag="bconst", name="bconst")
        nc.vector.memset(bconst[:], BSH)

        acc = [
            pool.tile([P, F], dt.float16, tag=f"acc{c}", name=f"acc{c}")
            for c in range(4)
        ]
        # multi-buffered mv tiles so producers run ahead of consumers
        NMV = 4
        mv = [
            pool.tile([P, FS], dt.float16, tag=f"mv{i}", name=f"mv{i}")
            for i in range(NMV * 4)
        ]
        amv = [
            pool.tile([P, FS], dt.float16, tag=f"amv{i}", name=f"amv{i}")
            for i in range(3 * 4)
        ]
        sqt = [
            pool.tile([P, FS], dt.float16, tag=f"sq{i}", name=f"sq{i}")
            for i in range(3)
        ]
        mvb = [
            pool.tile([P, FS], dt.float16, tag=f"mvb{s}", name=f"mvb{s}")
            for s in range(SC)
        ]
        qm = [
            pool.tile([P, FS], dt.float16, tag=f"qm{i}", name=f"qm{i}")
            for i in range(4 * SC)
        ]
        gprod = [
            pool.tile([P, FS], dt.float16, tag=f"gp{i}", name=f"gp{i}")
            for i in range(2)
        ]
        gjunk = [
            pool.tile([P, FS], dt.float16, tag=f"gj{i}", name=f"gj{i}")
            for i in range(2)
        ]
        gcell = pool.tile([P, 4 * SC], dt.float32, tag="gcell", name="gcell")

        # engine choice for accumulate adds: gpsimd takes a tuned share.
        # (class_tag, comp) -> engine;  class tags: 1..18 pos, "w0","w1",
        # "sub","em"
        def add_eng(tag, comp):
            if comp == 3:
                return nc.gpsimd
            if comp == 2 and isinstance(tag, int) and tag >= 14:
                return nc.gpsimd
            return nc.vector

        per_sc_results = []
        for s in range(SC):
            sl = slice(s * FS, (s + 1) * FS)
            p_s = p_t[:, sl]
            w_s = w_t[:, sl]
            m_s = msel_t[:, sl]
            acc_s = [a[:, sl] for a in acc]

            # init: class 0 writes (p==0)*V
            for c in range(4):
                nc.vector.tensor_scalar(
                    acc_s[c], p_s, 0.0, V(COL_POS + c), ALU.is_equal, ALU.mult
                )
            # DVE position classes 1..ACT_LO-1: fused mv + add
            for k in range(1, ACT_LO):
                for c in range(4):
                    m = mv[(k % NMV) * 4 + c]
                    nc.vector.tensor_scalar(
                        m[:], p_s, float(k), V(COL_POS + 4 * k + c),
                        ALU.is_equal, ALU.mult,
                    )
                    add_eng(k, c).tensor_add(acc_s[c], acc_s[c], m[:])
            # ACT position classes ACT_LO..18: Square once, 4 Relu mvs
            for ki, k in enumerate(range(ACT_LO, NPOS)):
                sq = sqt[ki % 3]
                nc.scalar.activation(sq[:], p_s, AF.Square, bias=bias_c[k][:])
                for c in range(4):
                    m = amv[(ki % 3) * 4 + c]
                    nc.scalar.activation(
                        m[:], sq[:], AF.Relu,
                        bias=V(COL_VB + 4 * k + c),
                        scale=V(COL_NVB + 4 * k + c),
                    )
                    add_eng(k, c).tensor_add(acc_s[c], acc_s[c], m[:])
            # remove the spurious +B over [p >= ACT_LO] (includes the single
            # t=0 pad step p=19; host adds B back to both outputs)
            nc.vector.tensor_scalar(
                mvb[s][:], p_s, ACT_LO - 0.5, bconst[:], ALU.is_gt, ALU.mult
            )
            for c in range(4):
                add_eng("sub", c).tensor_sub(acc_s[c], acc_s[c], mvb[s][:])
            # who2who classes
            for v in range(2):
                for c in range(4):
                    m = mv[((NPOS + v) % NMV) * 4 + c]
                    nc.vector.tensor_scalar(
                        m[:], w_s, float(v), V(COL_W + 4 * v + c),
                        ALU.is_equal, ALU.mult,
                    )
                    add_eng(f"w{v}", c).tensor_add(acc_s[c], acc_s[c], m[:])
            # emission fold: acc[i,j] += em_j
            for c in range(4):
                em_s = (em0_t if c % 2 == 0 else em1_t)[:, sl]
                add_eng("em", c).tensor_add(acc_s[c], acc_s[c], em_s)

            # ---- gold: sum_t acc[msel_t][t] ----
            for pair in range(4):
                q = qm[s * 4 + pair]
                nc.vector.tensor_scalar(q[:], m_s, float(pair), None,
                                        ALU.is_equal)
                pr = gprod[pair % 2]
                nc.vector.tensor_mul(pr[:], q[:], acc_s[pair])
                nc.vector.tensor_scalar(
                    gjunk[pair % 2][:], pr[:], 1.0, None, ALU.mult, ALU.add,
                    accum_out=gcell[:, s * 4 + pair:s * 4 + pair + 1],
                )

            # ---- in-partition tree over this sub-chunk ----
            HS = FS // 2
            FP16_LEVELS = 4
            X16 = pool.tile([P, HS, 2, 2], dt.float16, tag=f"X16s{s}",
                            name=f"X16s{s}")
            Y16a = pool.tile([P, HS, 2, 2], dt.float16, tag=f"Y16as{s}",
                             name=f"Y16as{s}")
            Y16b = pool.tile([P, HS // 2, 2, 2], dt.float16, tag=f"Y16bs{s}",
                             name=f"Y16bs{s}")
            X32 = pool.tile([P, HS // 16, 2, 2], dt.float32, tag=f"X32s{s}",
                            name=f"X32s{s}")
            Y0 = pool.tile([P, HS, 2, 2], dt.float32, tag=f"Y0s{s}",
                           name=f"Y0s{s}")
            Y1 = pool.tile([P, HS // 2, 2, 2], dt.float32, tag=f"Y1s{s}",
                           name=f"Y1s{s}")

            def u2(ap):
                return ap.unsqueeze(2).unsqueeze(3)

            for i in range(2):
                for j in range(2):
                    nc.vector.tensor_add(
                        X16[:, :, i:i + 1, j:j + 1],
                        u2(acc_s[_comp(i, 0)][:, 0::2]),
                        u2(acc_s[_comp(0, j)][:, 1::2]),
                    )
                    nc.vector.tensor_add(
                        Y16a[:, :, i:i + 1, j:j + 1],
                        u2(acc_s[_comp(i, 1)][:, 0::2]),
                        u2(acc_s[_comp(1, j)][:, 1::2]),
                    )
            nc.vector.tensor_sub(Y16a[:], Y16a[:], X16[:])
            nc.scalar.activation(Y0[:], Y16a[:], AF.Exp)
            nc.scalar.activation(Y0[:], Y0[:], AF.Ln, bias=1.0)
            mlev = pool.tile([P, HS, 2, 2], dt.float16, tag=f"m1s{s}",
                             name=f"m1s{s}")
            nc.vector.tensor_add(mlev[:], X16[:], Y0[:])

            w_cur = HS
            lev = 1
            while w_cur > 1:
                w2 = w_cur // 2
                lev += 1
                sh = [P, w2, 2, 2]
                a_i0 = mlev[:, 0:w_cur:2, :, 0:1].broadcast_to(sh)
                a_i1 = mlev[:, 0:w_cur:2, :, 1:2].broadcast_to(sh)
                b_0j = mlev[:, 1:w_cur:2, 0:1, :].broadcast_to(sh)
                b_1j = mlev[:, 1:w_cur:2, 1:2, :].broadcast_to(sh)
                sp = (Y0 if lev % 2 == 1 else Y1)[:, 0:w2]
                if lev <= FP16_LEVELS:
                    xv = X16[:, 0:w2]
                    yv = (Y16a if lev % 2 == 1 else Y16b)[:, 0:w2]
                    nc.vector.tensor_add(xv, a_i0, b_0j)
                    nc.vector.tensor_add(yv, a_i1, b_1j)
                    nc.vector.tensor_sub(yv, yv, xv)
                    nc.scalar.activation(sp, yv, AF.Exp)
                else:
                    xv = X32[:, 0:w2]
                    yv = sp
                    nc.vector.tensor_add(xv, a_i0, b_0j)
                    nc.vector.tensor_add(yv, a_i1, b_1j)
                    nc.vector.tensor_sub(yv, yv, xv)
                    nc.scalar.activation(sp, sp, AF.Exp)
                nc.scalar.activation(sp, sp, AF.Ln, bias=1.0)
                mdt = dt.float16 if lev <= FP16_LEVELS else dt.float32
                mnext = pool.tile(sh, mdt, tag=f"m{lev}s{s}", name=f"m{lev}s{s}")
                nc.vector.tensor_add(mnext[:], xv, sp)
                mlev = mnext
                w_cur = w2
            per_sc_results.append(mlev)

        # ---- combine the SC per-partition matrices (in step order) ----
        A, Bm = per_sc_results
        shc = [P, 1, 2, 2]
        CX = pool.tile(shc, dt.float32, tag="CX", name="CX")
        CY = pool.tile(shc, dt.float32, tag="CY", name="CY")
        nc.vector.tensor_add(CX[:], A[:, :, :, 0:1].broadcast_to(shc),
                             Bm[:, :, 0:1, :].broadcast_to(shc))
        nc.vector.tensor_add(CY[:], A[:, :, :, 1:2].broadcast_to(shc),
                             Bm[:, :, 1:2, :].broadcast_to(shc))
        nc.vector.tensor_sub(CY[:], CY[:], CX[:])
        nc.scalar.activation(CY[:], CY[:], AF.Exp)
        nc.scalar.activation(CY[:], CY[:], AF.Ln, bias=1.0)
        mfin = pool.tile(shc, dt.float32, tag="mfin", name="mfin")
        nc.vector.tensor_add(mfin[:], CX[:], CY[:])

        gold_part = pool.tile([P, 1], dt.float32, tag="gold_part",
                              name="gold_part")
        nc.vector.reduce_sum(gold_part[:], gcell[:], axis=AX.X)

        # ---- pack per-partition results and bounce through DRAM ----
        pk = pool.tile([P, E], dt.float32, tag="pk", name="pk")
        nc.vector.tensor_copy(
            pk[:, 0:4].rearrange("p (a b c) -> p a b c", a=1, b=2), mfin[:]
        )
        nc.vector.tensor_copy(pk[:, 4:5], gold_part[:])
        scr = dpool.tile([P, E], dt.float32, tag="scr", name="scr")
        nc.sync.dma_start(scr[:], pk[:])
        fin = pool.tile([1, P * E], dt.float32, tag="fin", name="fin")
        nc.sync.dma_start(fin[:], scr[:].rearrange("p e -> (p e)").unsqueeze(0))
        v = fin[:].rearrange("o (p e) -> o p e", e=E)

        gold_tot = pool.tile([1, 1], dt.float32, tag="gold_tot", name="gold_tot")
        nc.vector.reduce_sum(gold_tot[:], v[:, :, 4], axis=AX.X)

        # ---- tail tree over the 128 per-partition chunk matrices ----
        TX = pool.tile([1, P // 2, 2, 2], dt.float32, tag="TX", name="TX")
        TY0 = pool.tile([1, P // 2, 2, 2], dt.float32, tag="TY0", name="TY0")
        TY1 = pool.tile([1, P // 2, 2, 2], dt.float32, tag="TY1", name="TY1")
        w2 = P // 2
        sh = [1, w2, 2, 2]
        a_i0 = v[:, 0::2, 0:3:2].unsqueeze(3).broadcast_to(sh)
        a_i1 = v[:, 0::2, 1:4:2].unsqueeze(3).broadcast_to(sh)
        b_0j = v[:, 1::2, 0:2].unsqueeze(2).broadcast_to(sh)
        b_1j = v[:, 1::2, 2:4].unsqueeze(2).broadcast_to(sh)
        xv = TX[:, 0:w2]
        yv = TY0[:, 0:w2]
        nc.vector.tensor_add(xv, a_i0, b_0j)
        nc.vector.tensor_add(yv, a_i1, b_1j)
        nc.vector.tensor_sub(yv, yv, xv)
        nc.scalar.activation(yv, yv, AF.Exp)
        nc.scalar.activation(yv, yv, AF.Ln, bias=1.0)
        tlev = pool.tile(sh, dt.float32, tag="t1", name="t1")
        nc.vector.tensor_add(tlev[:], xv, yv)
        w_cur = w2
        lev = 1
        while w_cur > 1:
            w2 = w_cur // 2
            lev += 1
            sh = [1, w2, 2, 2]
            a_i0 = tlev[:, 0:w_cur:2, :, 0:1].broadcast_to(sh)
            a_i1 = tlev[:, 0:w_cur:2, :, 1:2].broadcast_to(sh)
            b_0j = tlev[:, 1:w_cur:2, 0:1, :].broadcast_to(sh)
            b_1j = tlev[:, 1:w_cur:2, 1:2, :].broadcast_to(sh)
            xv = TX[:, 0:w2]
            yv = (TY0 if lev % 2 == 1 else TY1)[:, 0:w2]
            nc.vector.tensor_add(xv, a_i0, b_0j)
            nc.vector.tensor_add(yv, a_i1, b_1j)
            nc.vector.tensor_sub(yv, yv, xv)
            nc.scalar.activation(yv, yv, AF.Exp)
            nc.scalar.activation(yv, yv, AF.Ln, bias=1.0)
            tnext = pool.tile(sh, dt.float32, tag=f"t{lev}", name=f"t{lev}")
            nc.vector.tensor_add(tnext[:], xv, yv)
            tlev = tnext
            w_cur = w2

        # ---- assemble [P00, P01, P10, P11, gold, 0, 0, 0] and store ----
        res = pool.tile([1, 8], dt.float32, tag="res", name="res")
        nc.vector.memset(res[:], 0.0)
        nc.vector.tensor_copy(
            res[:, 0:4].rearrange("p (a b c) -> p a b c", a=1, b=2), tlev[:]
        )
        nc.vector.tensor_copy(res[:, 4:5], gold_tot[:])
        nc.sync.dma_start(out_d[:], res[:])

    nc.compile()

    # Both Exp and Ln live in the 'natural_log_exp_and_others' ACT table set
    # (which also holds Square/Relu used by the ACT-produced mvs), but
    # insert_act_table_loads picks the first set containing each function,
    # emitting alternating table reloads (1.3 us each).  Retarget every load
    # to the combined set and drop the now-redundant ones.
    from concourse.hw_specs import get_activation_tables

    tables = list(get_activation_tables(nc.m.arch).keys())
    combined = tables.index("natural_log_exp_and_others")
    for b in nc.bb_map.values():
        insts = b.bb.instructions
        kept = []
        seen_load = False
        for ins in insts:
            if ins.opcode == "LoadActFuncSet":
                si = ins.sync_info
                assert not (si and (si.on_wait or si.on_update)), ins.name
                if seen_load:
                    continue
                ins.act_func_set_id = combined
                seen_load = True
            kept.append(ins)
        if len(kept) != len(insts):
            b.bb.instructions = kept
    return nc


def _get_nc():
    global _NC_CACHE
    if _NC_CACHE is None:
        _NC_CACHE = _build_nc()
    return _NC_CACHE


def _f16_candidates(x, grid_pow=None):
    """Nearest fp16 (or 2^grid_pow-grid) value and its other-side neighbor."""
    if grid_pow is None:
        lo = np.float16(x)
        res = float(x) - float(lo)
        if res == 0.0:
            return np.float32(lo), np.float32(lo)
        hi = np.nextafter(lo, np.float16(np.inf if res > 0 else -np.inf),
                          dtype=np.float16)
        return np.float32(lo), np.float32(hi)
    g = 2.0 ** grid_pow
    lo = np.floor(float(x) / g) * g
    hi = lo + g
    if abs(float(x) - lo) <= abs(hi - float(x)):
        return np.float32(lo), np.float32(hi)
    return np.float32(hi), np.float32(lo)


def _optimize_tables(pos, w2w):
    """fp16 tables with per-entry rounding direction chosen so the
    systematic selected-sum bias over the reachable (p,w) cells cancels.
    ACT-suffix position entries are kept on the 2^-8 grid so V+BSH stays
    fp16-exact."""
    Pc = np.zeros((NPOS, 4, 2), np.float32)
    for k in range(NPOS):
        gp = -8 if k >= ACT_LO else None
        for c in range(4):
            Pc[k, c] = _f16_candidates(pos.reshape(NPOS, 4)[k, c], gp)
    Wc = np.zeros((2, 4, 2), np.float32)
    for v in range(2):
        for c in range(4):
            Wc[v, c] = _f16_candidates(w2w.reshape(2, 4)[v, c])

    exact = (pos.reshape(NPOS, 1, 4).astype(np.float64)
             + w2w.reshape(1, 2, 4).astype(np.float64))  # [19,2,4]
    # delta[k,v,c,pk,wv] = f16(Pc+Wc) - exact
    d = (Pc[:, None, :, :, None].astype(np.float16)
         + Wc[None, :, :, None, :].astype(np.float16)).astype(np.float16)
    delta = d.astype(np.float64) - exact[:, :, :, None, None]

    Ps = np.zeros((NPOS, 4), np.intp)
    Ws = np.zeros((2, 4), np.intp)

    def total():
        s = 0.0
        for k in range(NPOS):
            for v in range(2):
                for c in range(4):
                    s += delta[k, v, c, Ps[k, c], Ws[v, c]]
        return s

    best = total()
    for _ in range(4):
        improved = False
        for k in range(NPOS):
            for c in range(4):
                Ps[k, c] ^= 1
                t2 = total()
                if abs(t2) < abs(best):
                    best = t2
                    improved = True
                else:
                    Ps[k, c] ^= 1
        for v in range(2):
            for c in range(4):
                Ws[v, c] ^= 1
                t2 = total()
                if abs(t2) < abs(best):
                    best = t2
                    improved = True
                else:
                    Ws[v, c] ^= 1
        if not improved:
            break

    P16 = np.take_along_axis(Pc, Ps[:, :, None], axis=2)[:, :, 0]
    W16 = np.take_along_axis(Wc, Ws[:, :, None], axis=2)[:, :, 0]
    return P16.astype(np.float32), W16.astype(np.float32)


def kernel(**inputs):
    em = np.asarray(inputs["emission_scores"], dtype=np.float32)
    lab = np.asarray(inputs["label"]).astype(np.float32)
    w = np.asarray(inputs["who2who_state"]).astype(np.float32)
    p = np.asarray(inputs["position_state"]).astype(np.float32)
    w2w = np.asarray(inputs["who2who_params"], dtype=np.float32)
    pos = np.asarray(inputs["position_params"], dtype=np.float32)
    assert em.shape == (T, 2), em.shape

    labp = np.empty_like(lab)
    labp[0] = 0.0
    labp[1:] = lab[:-1]
    msel = (2.0 * labp + lab).astype(np.float16)

    P16, W16 = _optimize_tables(pos, w2w)
    vb = P16 + np.float32(BSH)   # fp16-exact for the ACT rows (2^-8 grid)
    par_row = np.concatenate([
        P16.reshape(-1), W16.reshape(-1), vb.reshape(-1), (-vb).reshape(-1)
    ]).astype(np.float32)
    assert par_row.shape[0] == NPAR
    par16 = np.broadcast_to(par_row.view(np.float16), (P, 2 * NPAR))

    p16 = p.astype(np.float16)
    w16 = w.astype(np.float16)
    em16 = em.astype(np.float16)

    in_maps = []
    for k in range(NCORES):
        sl = slice(k * L, (k + 1) * L)
        blob0 = np.concatenate(
            [
                par16,
                p16[sl].reshape(P, F),
                w16[sl].reshape(P, F),
                msel[sl].reshape(P, F),
                np.ascontiguousarray(em16[sl, 0].reshape(P, F)),
                np.ascontiguousarray(em16[sl, 1].reshape(P, F)),
            ],
            axis=1,
        )
        in_maps.append({"blob0": np.ascontiguousarray(blob0)})

    nc = _get_nc()
    kr = bass_utils.run_bass_kernel_spmd(nc, in_maps, core_ids=list(range(NCORES)))
    global LAST_RESULTS
    LAST_RESULTS = kr
    results = kr.results

    # host combine: 7 log-semiring 2x2 products (in order) + gold partial sum
    mats = []
    gold = 0.0
    for r in results:
        row = np.asarray(r["out"], dtype=np.float64).reshape(-1)
        mats.append(row[0:4].reshape(2, 2))
        gold += row[4]
    U = mats[0]
    for M in mats[1:]:
        U = np.logaddexp(U[:, 0:1] + M[0:1, :], U[:, 1:2] + M[1:2, :])
    total = np.logaddexp.reduce(U.reshape(-1))
    # the single t=0 pad step (p=19) carries the -BSH shift: add it back
    gold += BSH
    total += BSH
    return np.stack([gold, total]).astype(np.float32)


if __name__ == "__main__":
    rng = np.random.default_rng(0)
    demo = dict(
        emission_scores=rng.standard_normal((T, 2)).astype(np.float32),
        label=rng.integers(0, 2, T),
        who2who_state=np.concatenate([[2], rng.integers(0, 2, T - 1)]),
        position_state=np.concatenate([[19], rng.integers(0, 19, T - 1)]),
        who2who_params=rng.standard_normal((2, 2, 2)).astype(np.float32),
        position_params=rng.standard_normal((19, 2, 2)).astype(np.float32),
    )
    print(kernel(**demo))


# revision 11
# speedup vs baseline: 1.2024x; 1.0958x over previous
"""Trainium2 Bass kernel for a 2-state linear-chain CRF loss (BiLSTM-CRF loss_fn).

Computes, for a single conversation of length T = 2,097,152:
  gold_score  = sum_t em[t, lab[t]] + sum_{t>0} trans[t][lab[t-1], lab[t]]
  total_score = logsumexp of the CRF forward recursion
where trans[t] = who2who_sub[w[t]] + position_sub[p[t]] (60 possible 2x2
matrices; indices 2/19 select an all-zero padding matrix).

Design (one NeuronCore per contiguous chunk of 262,144 steps, 8 cores):

* Per-step matrices: trans+em is built as 4 fp16 streams by per-class masked
  accumulation (19 position classes + 2 who2who classes + emission fold).
  Class supports are disjoint, so sums of masked values are exact in fp16;
  the masked values are combined PAIRWISE (a small in-group tree) so the
  per-stream dependency depth is ~8 instead of 21 serial adds.  Work is
  split three ways: DVE runs fused (idx==c)*V tensor_scalars (4x fp16 mode)
  plus most adds; the ACT engine produces masked values for a suffix of
  position classes as Relu((V+B) - (V+B)*(p-c)^2) with B=4 making the peak
  positive (the spurious +B*[p>=a] is removed by one (p>a-.5)*B mask and 4
  subtracts; the t=0 pad step ends shifted by exactly -B, corrected on the
  host); GPSIMD takes a striped share of the adds.

* Gold score: the label-pair stream msel = 2*lab[t-1]+lab[t] selects one of
  the 4 finished streams per step; gold = sum_t acc[msel_t][t] via 4
  is_equal masks + multiply + accum_out per sub-chunk.  Exactness: stream
  values are single-fp16-rounded table values (+ exact-in-fp16 shifts), and
  the host chooses each table entry's fp16 rounding DIRECTION (greedy sign
  optimization over the 19x2x4 reachable cells) so the systematic selection
  bias cancels to ~1e-5 relative.

* Forward pass: the recursion is a product of 2x2 matrices in the (log, +)
  semiring; each core tree-reduces with LSE(a,b) = a + ln(1+exp(b-a)) on
  ACT.  The chunk is split into 3 sub-chunks of 1024/512/512 steps per
  partition so each sub-chunk's tree overlaps the next one's stream build
  and only the last (small) tree is exposed at the end.  Each core ships
  its 3*128 sub-chunk matrices + per-partition gold; the host does the
  O(cores*P) ordered log-semiring combine (vectorized numpy).

* All inputs ship as one fp16 blob [par | p | w | msel | em0 | em1] in 3
  DMAs so the class masks start immediately.
"""

from contextlib import ExitStack

import numpy as np

import concourse.bass as bass
import concourse.bacc as bacc
import concourse.mybir as mybir
import concourse.tile as tile
from concourse import bass_utils

dt = mybir.dt
ALU = mybir.AluOpType
AF = mybir.ActivationFunctionType
AX = mybir.AxisListType

T = 2097152
NCORES = 8
P = 128                  # SBUF partitions
L = T // NCORES          # steps per core = 262144
F = L // P               # steps per partition = 2048
SC_SIZES = (1024, 512, 512)
SC = len(SC_SIZES)
NPOS = 19                # position classes with nonzero matrices
BSH = 4.0                # ACT positivity shift
ACT_LO = 9               # position classes >= this use ACT-produced mv
EW = 4 * SC + 1          # out row: SC matrices (4 entries each) + gold

# param row layout (f32 words): [pos 19*4 | w2w 2*4 | VB 19*4 | negVB 19*4]
NPAR = 19 * 4 + 8 + 19 * 4 + 19 * 4
COL_POS = 0
COL_W = 76
COL_VB = 84
COL_NVB = 160

W0 = 2 * NPAR + 5 * F    # fp16 blob columns


_NC_CACHE = None
LAST_RESULTS = None  # BassKernelResults of the most recent kernel() call


def _comp(i, j):
    return i * 2 + j


def _build_nc():
    nc = bacc.Bacc()

    b0_d = nc.dram_tensor("blob0", [P, W0], dt.float16, kind="ExternalInput")
    out_d = nc.dram_tensor("out", [P, EW], dt.float32, kind="ExternalOutput")

    with ExitStack() as ctx:
        tc = ctx.enter_context(tile.TileContext(nc))
        pool = ctx.enter_context(tc.tile_pool(name="main", bufs=1))

        # ---- loads: [par | p | w | msel | em0 | em1] in 3 DMAs ----
        b0 = pool.tile([P, W0], dt.float16, tag="b0", name="b0")
        h0 = 2 * NPAR + F          # par + p
        h1 = h0 + 2 * F            # + w + msel
        nc.sync.dma_start(b0[:, 0:h0], b0_d[:, 0:h0])
        nc.sync.dma_start(b0[:, h0:h1], b0_d[:, h0:h1])
        nc.sync.dma_start(b0[:, h1:W0], b0_d[:, h1:W0])

        par32 = b0[:, 0:2 * NPAR].bitcast(dt.float32)
        p_t = b0[:, 2 * NPAR:h0]
        w_t = b0[:, h0:h0 + F]
        msel_t = b0[:, h0 + F:h1]
        em0_t = b0[:, h1:h1 + F]
        em1_t = b0[:, h1 + F:W0]

        def V(col):
            return par32[:, col:col + 1]

        bias_c = {}
        for c in range(ACT_LO, NPOS):
            t_ = pool.tile([P, 1], dt.float32, tag=f"bc{c}", name=f"bc{c}")
            nc.vector.memset(t_[:], -float(c))
            bias_c[c] = t_
        bconst = pool.tile([P, 1], dt.float32, tag="bconst", name="bconst")
        nc.vector.memset(bconst[:], BSH)

        FSMAX = max(SC_SIZES)
        HSMAX = FSMAX // 2
        SC_OFF = [sum(SC_SIZES[:i]) for i in range(SC)]
        acc = [
            pool.tile([P, F], dt.float16, tag=f"acc{c}", name=f"acc{c}")
            for c in range(4)
        ]
        # mv work tiles, full-F, sliced per sub-chunk (cross-SC skew spaces
        # same-SC reuse)
        mv = [
            pool.tile([P, F], dt.float16, tag=f"mv{i}", name=f"mv{i}")
            for i in range(4 * 4)
        ]

        def mv_t(ki, c, s):
            return mv[ki * 4 + c][:, SC_OFF[s]:SC_OFF[s] + SC_SIZES[s]]

        amv = [
            pool.tile([P, F], dt.float16, tag=f"amv{i}", name=f"amv{i}")
            for i in range(3 * 4)
        ]

        def amv_t(ki, c, s):
            return amv[ki * 4 + c][:, SC_OFF[s]:SC_OFF[s] + SC_SIZES[s]]

        sqt = [
            pool.tile([P, F], dt.float16, tag=f"sq{i}", name=f"sq{i}")
            for i in range(2)
        ]
        mvb = pool.tile([P, FSMAX], dt.float16, tag="mvb", name="mvb")
        qm = [
            pool.tile([P, FSMAX], dt.float16, tag=f"qm{i}", name=f"qm{i}")
            for i in range(2)
        ]
        gprod = [
            pool.tile([P, FSMAX], dt.float16, tag=f"gp{i}", name=f"gp{i}")
            for i in range(2)
        ]
        gcell = pool.tile([P, 4 * SC], dt.float32, tag="gcell", name="gcell")
        res = pool.tile([P, EW], dt.float32, tag="res", name="res")

        # striped DVE/Pool assignment for accumulate adds
        POOL_NUM, POOL_DEN = 1, 3
        add_ctr = [0] * 4

        def add_eng(comp):
            add_ctr[comp] += 1
            k = (add_ctr[comp] + comp) % POOL_DEN
            return nc.gpsimd if k < POOL_NUM else nc.vector

        def tadd(comp, out, a, b):
            add_eng(comp).tensor_add(out, a, b)

        def sc_views(s):
            sl = slice(SC_OFF[s], SC_OFF[s] + SC_SIZES[s])
            return (p_t[:, sl], w_t[:, sl], msel_t[:, sl],
                    [a[:, sl] for a in acc], sl)

        dve_classes = list(range(0, ACT_LO))
        dgroups = [dve_classes[i:i + 4]
                   for i in range(0, len(dve_classes), 4)]
        act_classes = list(range(ACT_LO, NPOS))
        agroups = [act_classes[i:i + 3]
                   for i in range(0, len(act_classes), 3)]

        def emit_dve_group(s, gi):
            grp = dgroups[gi]
            p_s, w_s, m_s, acc_s, sl = sc_views(s)
            FS = SC_SIZES[s]
            for c in range(4):
                tiles = []
                for ki, k in enumerate(grp):
                    m = mv_t(ki, c, s)
                    nc.vector.tensor_scalar(
                        m, p_s, float(k), V(COL_POS + 4 * k + c),
                        ALU.is_equal, ALU.mult,
                    )
                    tiles.append(m)
                if len(tiles) == 4:
                    tadd(c, tiles[0], tiles[0], tiles[1])
                    tadd(c, tiles[2], tiles[2], tiles[3])
                    if gi == 0:
                        tadd(c, acc_s[c], tiles[0], tiles[2])
                    else:
                        tadd(c, tiles[0], tiles[0], tiles[2])
                        tadd(c, acc_s[c], acc_s[c], tiles[0])
                else:
                    while len(tiles) > 1:
                        tadd(c, tiles[0], tiles[0], tiles[1])
                        tiles = [tiles[0]] + tiles[2:]
                    if gi == 0:
                        nc.vector.tensor_copy(acc_s[c], tiles[0])
                    else:
                        tadd(c, acc_s[c], acc_s[c], tiles[0])

        def emit_act_group(s, gi):
            grp = agroups[gi]
            p_s, w_s, m_s, acc_s, sl = sc_views(s)
            FS = SC_SIZES[s]
            for ki, k in enumerate(grp):
                sq = sqt[ki % 2][:, SC_OFF[s]:SC_OFF[s] + FS]
                nc.scalar.activation(sq, p_s, AF.Square, bias=bias_c[k][:])
                for c in range(4):
                    nc.scalar.activation(
                        amv_t(ki, c, s), sq, AF.Relu,
                        bias=V(COL_VB + 4 * k + c),
                        scale=V(COL_NVB + 4 * k + c),
                    )
            for c in range(4):
                tiles = [amv_t(ki, c, s) for ki in range(len(grp))]
                while len(tiles) > 1:
                    tadd(c, tiles[0], tiles[0], tiles[1])
                    tiles = [tiles[0]] + tiles[2:]
                tadd(c, acc_s[c], acc_s[c], tiles[0])

        def emit_sub_w_em(s):
            p_s, w_s, m_s, acc_s, sl = sc_views(s)
            FS = SC_SIZES[s]
            # remove the spurious +B over [p >= ACT_LO] (includes the t=0
            # pad step p=19; host adds B back to both outputs)
            nc.vector.tensor_scalar(
                mvb[:, 0:FS], p_s, ACT_LO - 0.5, bconst[:], ALU.is_gt,
                ALU.mult,
            )
            for c in range(4):
                add_eng(c).tensor_sub(acc_s[c], acc_s[c], mvb[:, 0:FS])
            # who2who classes: pair the two masked values, then one link
            for c in range(4):
                m0 = mv_t(0, c, s)
                m1 = mv_t(1, c, s)
                for v, m in ((0, m0), (1, m1)):
                    nc.vector.tensor_scalar(
                        m, w_s, float(v), V(COL_W + 4 * v + c),
                        ALU.is_equal, ALU.mult,
                    )
                tadd(c, m0, m0, m1)
                tadd(c, acc_s[c], acc_s[c], m0)
            # emission fold: acc[i,j] += em_j
            for c in range(4):
                em_s = (em0_t if c % 2 == 0 else em1_t)[:, sl]
                tadd(c, acc_s[c], acc_s[c], em_s)

        def emit_gold(s):
            p_s, w_s, m_s, acc_s, sl = sc_views(s)
            FS = SC_SIZES[s]
            for pair in range(4):
                q = qm[pair % 2][:, 0:FS]
                nc.vector.tensor_scalar(q, m_s, float(pair), None,
                                        ALU.is_equal)
                pr = gprod[pair % 2][:, 0:FS]
                nc.vector.tensor_mul(pr, q, acc_s[pair])
                nc.vector.tensor_scalar(
                    pr, pr, 1.0, None, ALU.mult, ALU.add,
                    accum_out=gcell[:, s * 4 + pair:s * 4 + pair + 1],
                )

        def u2(ap):
            return ap.unsqueeze(2).unsqueeze(3)

        def emit_tree(s):
            p_s, w_s, m_s, acc_s, sl = sc_views(s)
            FS = SC_SIZES[s]
            HS = FS // 2
            FP16_LEVELS = 4
            X16 = pool.tile([P, HSMAX, 2, 2], dt.float16, tag="X16",
                            name="X16")[:, 0:HS]
            Y16a = pool.tile([P, HSMAX, 2, 2], dt.float16, tag="Y16a",
                             name="Y16a")[:, 0:HS]
            Y16b = pool.tile([P, HSMAX // 2, 2, 2], dt.float16, tag="Y16b",
                             name="Y16b")[:, 0:HS // 2]
            X32 = pool.tile([P, HSMAX // 16, 2, 2], dt.float32, tag="X32",
                            name="X32")[:, 0:max(HS // 16, 1)]
            Y0 = pool.tile([P, HSMAX, 2, 2], dt.float32, tag="Y0",
                           name="Y0")[:, 0:HS]
            Y1 = pool.tile([P, HSMAX // 2, 2, 2], dt.float32, tag="Y1",
                           name="Y1")[:, 0:HS // 2]
            for i in range(2):
                for j in range(2):
                    nc.vector.tensor_add(
                        X16[:, :, i:i + 1, j:j + 1],
                        u2(acc_s[_comp(i, 0)][:, 0::2]),
                        u2(acc_s[_comp(0, j)][:, 1::2]),
                    )
                    nc.vector.tensor_add(
                        Y16a[:, :, i:i + 1, j:j + 1],
                        u2(acc_s[_comp(i, 1)][:, 0::2]),
                        u2(acc_s[_comp(1, j)][:, 1::2]),
                    )
            nc.vector.tensor_sub(Y16a[:], Y16a[:], X16[:])
            nc.scalar.activation(Y0[:], Y16a[:], AF.Exp)
            nc.scalar.activation(Y0[:], Y0[:], AF.Ln, bias=1.0)
            mlev = pool.tile([P, HSMAX, 2, 2], dt.float16, tag="m1",
                             name="m1")[:, 0:HS]
            nc.vector.tensor_add(mlev[:], X16[:], Y0[:])

            w_cur = HS
            lev = 1
            while w_cur > 1:
                w2 = w_cur // 2
                lev += 1
                sh = [P, w2, 2, 2]
                a_i0 = mlev[:, 0:w_cur:2, :, 0:1].broadcast_to(sh)
                a_i1 = mlev[:, 0:w_cur:2, :, 1:2].broadcast_to(sh)
                b_0j = mlev[:, 1:w_cur:2, 0:1, :].broadcast_to(sh)
                b_1j = mlev[:, 1:w_cur:2, 1:2, :].broadcast_to(sh)
                sp = (Y0 if lev % 2 == 1 else Y1)[:, 0:w2]
                if lev <= FP16_LEVELS:
                    xv = X16[:, 0:w2]
                    yv = (Y16a if lev % 2 == 1 else Y16b)[:, 0:w2]
                    nc.vector.tensor_add(xv, a_i0, b_0j)
                    nc.vector.tensor_add(yv, a_i1, b_1j)
                    nc.vector.tensor_sub(yv, yv, xv)
                    nc.scalar.activation(sp, yv, AF.Exp)
                else:
                    xv = X32[:, 0:w2]
                    yv = sp
                    nc.vector.tensor_add(xv, a_i0, b_0j)
                    nc.vector.tensor_add(yv, a_i1, b_1j)
                    nc.vector.tensor_sub(yv, yv, xv)
                    nc.scalar.activation(sp, sp, AF.Exp)
                nc.scalar.activation(sp, sp, AF.Ln, bias=1.0)
                mdt = dt.float16 if lev <= FP16_LEVELS else dt.float32
                mwidth = max(HSMAX // (2 ** (lev - 1)), 1)
                mnext = pool.tile([P, mwidth, 2, 2], mdt, tag=f"m{lev}",
                                  name=f"m{lev}")[:, 0:w2]
                nc.vector.tensor_add(mnext[:], xv, sp)
                mlev = mnext
                w_cur = w2
            nc.vector.tensor_copy(
                res[:, 4 * s:4 * s + 4].rearrange("p (a b c) -> p a b c",
                                                  a=1, b=2),
                mlev[:],
            )

        # ---- skewed block emission: sub-chunk s trails s-1 by SKEW blocks
        # so completions stagger and each tree overlaps the next build ----
        SKEW = 4
        blocks = []
        for s in range(SC):
            seq = []
            seq.append(lambda s=s: emit_dve_group(s, 0))
            for gi in range(len(agroups)):
                seq.append(lambda s=s, gi=gi: emit_act_group(s, gi))
                if gi + 1 < len(dgroups):
                    seq.append(lambda s=s, gi=gi: emit_dve_group(s, gi + 1))
            seq.append(lambda s=s: emit_sub_w_em(s))
            seq.append(lambda s=s: emit_gold(s))
            seq.append(lambda s=s: emit_tree(s))
            for bi, fn in enumerate(seq):
                blocks.append((bi + SKEW * s, s, fn))
        blocks.sort(key=lambda kv: (kv[0], kv[1]))
        for _, _, fn in blocks:
            fn()

        # ---- gold column and store; host combines ----
        nc.vector.reduce_sum(res[:, 4 * SC:4 * SC + 1], gcell[:], axis=AX.X)
        nc.sync.dma_start(out_d[:], res[:])

    nc.compile()

    # Exp/Ln/Square/Relu all live in 'natural_log_exp_and_others', but
    # insert_act_table_loads picks the first set containing each function,
    # emitting alternating table reloads (1.3 us each).  Retarget every load
    # to the combined set and drop the now-redundant ones.
    from concourse.hw_specs import get_activation_tables

    tables = list(get_activation_tables(nc.m.arch).keys())
    combined = tables.index("natural_log_exp_and_others")
    for b in nc.bb_map.values():
        insts = b.bb.instructions
        kept = []
        seen_load = False
        for ins in insts:
            if ins.opcode == "LoadActFuncSet":
                si = ins.sync_info
                assert not (si and (si.on_wait or si.on_update)), ins.name
                if seen_load:
                    continue
                ins.act_func_set_id = combined
                seen_load = True
            kept.append(ins)
        if len(kept) != len(insts):
            b.bb.instructions = kept
    return nc


def _get_nc():
    global _NC_CACHE
    if _NC_CACHE is None:
        _NC_CACHE = _build_nc()
    return _NC_CACHE


def _f16_candidates(x, grid_pow=None):
    """Nearest fp16 (or 2^grid_pow-grid) value and its other-side neighbor."""
    if grid_pow is None:
        lo = np.float16(x)
        res = float(x) - float(lo)
        if res == 0.0:
            return np.float32(lo), np.float32(lo)
        hi = np.nextafter(lo, np.float16(np.inf if res > 0 else -np.inf),
                          dtype=np.float16)
        return np.float32(lo), np.float32(hi)
    g = 2.0 ** grid_pow
    lo = np.floor(float(x) / g) * g
    hi = lo + g
    if abs(float(x) - lo) <= abs(hi - float(x)):
        return np.float32(lo), np.float32(hi)
    return np.float32(hi), np.float32(lo)


def _optimize_tables(pos, w2w):
    """fp16 tables with per-entry rounding direction chosen so the
    systematic selected-sum bias over the reachable (p,w) cells cancels.
    ACT-suffix position entries are kept on the 2^-8 grid so V+BSH stays
    fp16-exact."""
    Pc = np.zeros((NPOS, 4, 2), np.float32)
    for k in range(NPOS):
        gp = -8 if k >= ACT_LO else None
        for c in range(4):
            Pc[k, c] = _f16_candidates(pos.reshape(NPOS, 4)[k, c], gp)
    Wc = np.zeros((2, 4, 2), np.float32)
    for v in range(2):
        for c in range(4):
            Wc[v, c] = _f16_candidates(w2w.reshape(2, 4)[v, c])

    exact = (pos.reshape(NPOS, 1, 4).astype(np.float64)
             + w2w.reshape(1, 2, 4).astype(np.float64))  # [19,2,4]
    d = (Pc[:, None, :, :, None].astype(np.float16)
         + Wc[None, :, :, None, :].astype(np.float16)).astype(np.float16)
    delta = d.astype(np.float64) - exact[:, :, :, None, None]

    Ps = np.zeros((NPOS, 4), np.intp)
    Ws = np.zeros((2, 4), np.intp)

    def total():
        s = 0.0
        for k in range(NPOS):
            for v in range(2):
                for c in range(4):
                    s += delta[k, v, c, Ps[k, c], Ws[v, c]]
        return s

    best = total()
    for _ in range(4):
        improved = False
        for k in range(NPOS):
            for c in range(4):
                Ps[k, c] ^= 1
                t2 = total()
                if abs(t2) < abs(best):
                    best = t2
                    improved = True
                else:
                    Ps[k, c] ^= 1
        for v in range(2):
            for c in range(4):
                Ws[v, c] ^= 1
                t2 = total()
                if abs(t2) < abs(best):
                    best = t2
                    improved = True
                else:
                    Ws[v, c] ^= 1
        if not improved:
            break

    P16 = np.take_along_axis(Pc, Ps[:, :, None], axis=2)[:, :, 0]
    W16 = np.take_along_axis(Wc, Ws[:, :, None], axis=2)[:, :, 0]
    return P16.astype(np.float32), W16.astype(np.float32)


def _lse_combine(A, B):
    """ordered log-semiring 2x2 product, vectorized over leading dims"""
    return np.logaddexp(A[..., :, 0:1] + B[..., 0:1, :],
                        A[..., :, 1:2] + B[..., 1:2, :])


def kernel(**inputs):
    em = np.asarray(inputs["emission_scores"], dtype=np.float32)
    lab = np.asarray(inputs["label"]).astype(np.float32)
    w = np.asarray(inputs["who2who_state"]).astype(np.float32)
    p = np.asarray(inputs["position_state"]).astype(np.float32)
    w2w = np.asarray(inputs["who2who_params"], dtype=np.float32)
    pos = np.asarray(inputs["position_params"], dtype=np.float32)
    assert em.shape == (T, 2), em.shape

    labp = np.empty_like(lab)
    labp[0] = 0.0
    labp[1:] = lab[:-1]
    msel = (2.0 * labp + lab).astype(np.float16)

    P16, W16 = _optimize_tables(pos, w2w)
    vb = P16 + np.float32(BSH)   # fp16-exact for the ACT rows (2^-8 grid)
    par_row = np.concatenate([
        P16.reshape(-1), W16.reshape(-1), vb.reshape(-1), (-vb).reshape(-1)
    ]).astype(np.float32)
    assert par_row.shape[0] == NPAR
    par16 = np.broadcast_to(par_row.view(np.float16), (P, 2 * NPAR))

    p16 = p.astype(np.float16)
    w16 = w.astype(np.float16)
    em16 = em.astype(np.float16)

    in_maps = []
    for k in range(NCORES):
        sl = slice(k * L, (k + 1) * L)
        blob0 = np.concatenate(
            [
                par16,
                p16[sl].reshape(P, F),
                w16[sl].reshape(P, F),
                msel[sl].reshape(P, F),
                np.ascontiguousarray(em16[sl, 0].reshape(P, F)),
                np.ascontiguousarray(em16[sl, 1].reshape(P, F)),
            ],
            axis=1,
        )
        in_maps.append({"blob0": np.ascontiguousarray(blob0)})

    nc = _get_nc()
    kr = bass_utils.run_bass_kernel_spmd(nc, in_maps, core_ids=list(range(NCORES)))
    global LAST_RESULTS
    LAST_RESULTS = kr
    results = kr.results

    # host combine: ordered product of NCORES*P*SC 2x2 matrices + gold sum
    rows = np.stack([np.asarray(r["out"], dtype=np.float64) for r in results])
    gold = rows[:, :, 4 * SC].sum()
    mats = rows[:, :, 0:4 * SC].reshape(NCORES * P * SC, 2, 2)
    # pairwise tree keeps it fast and stable
    while mats.shape[0] > 1:
        n = mats.shape[0]
        even = mats[0:n - 1:2]
        odd = mats[1:n:2]
        comb = _lse_combine(even, odd)
        if n % 2 == 1:
            comb = np.concatenate([comb, mats[n - 1:n]], axis=0)
        mats = comb
    total = np.logaddexp.reduce(mats.reshape(-1))
    # the single t=0 pad step (p=19) carries the -BSH shift: add it back
    gold += BSH
    total += BSH
    return np.stack([gold, total]).astype(np.float32)


if __name__ == "__main__":
    rng = np.random.default_rng(0)
    demo = dict(
        emission_scores=rng.standard_normal((T, 2)).astype(np.float32),
        label=rng.integers(0, 2, T),
        who2who_state=np.concatenate([[2], rng.integers(0, 2, T - 1)]),
        position_state=np.concatenate([[19], rng.integers(0, 19, T - 1)]),
        who2who_params=rng.standard_normal((2, 2, 2)).astype(np.float32),
        position_params=rng.standard_normal((19, 2, 2)).astype(np.float32),
    )
    print(kernel(**demo))


# revision 12
# speedup vs baseline: 1.2265x; 1.0200x over previous
"""Trainium2 Bass kernel for a 2-state linear-chain CRF loss (BiLSTM-CRF loss_fn).

Computes, for a single conversation of length T = 2,097,152:
  gold_score  = sum_t em[t, lab[t]] + sum_{t>0} trans[t][lab[t-1], lab[t]]
  total_score = logsumexp of the CRF forward recursion
where trans[t] = who2who_sub[w[t]] + position_sub[p[t]] (60 possible 2x2
matrices; indices 2/19 select an all-zero padding matrix).

Design (one NeuronCore per contiguous chunk of 262,144 steps, 8 cores):

* Per-step matrices: trans+em is built as 4 fp16 streams by per-class masked
  accumulation (19 position classes + 2 who2who classes + emission fold).
  Class supports are disjoint, so sums of masked values are exact in fp16;
  the masked values are combined PAIRWISE (a small in-group tree) so the
  per-stream dependency depth is ~8 instead of 21 serial adds.  Work is
  split three ways: DVE runs fused (idx==c)*V tensor_scalars (4x fp16 mode)
  plus most adds; the ACT engine produces masked values for a suffix of
  position classes as Relu((V+B) - (V+B)*(p-c)^2) with B=4 making the peak
  positive (the spurious +B*[p>=a] is removed by one (p>a-.5)*B mask and 4
  subtracts; the t=0 pad step ends shifted by exactly -B, corrected on the
  host); GPSIMD takes a striped share of the adds.

* Gold score: the label-pair stream msel = 2*lab[t-1]+lab[t] selects one of
  the 4 finished streams per step; gold = sum_t acc[msel_t][t] via 4
  is_equal masks + multiply + accum_out per sub-chunk.  Exactness: stream
  values are single-fp16-rounded table values (+ exact-in-fp16 shifts), and
  the host chooses each table entry's fp16 rounding DIRECTION (greedy sign
  optimization over the 19x2x4 reachable cells) so the systematic selection
  bias cancels to ~1e-5 relative.

* Forward pass: the recursion is a product of 2x2 matrices in the (log, +)
  semiring; each core tree-reduces with LSE(a,b) = a + ln(1+exp(b-a)) on
  ACT.  The chunk is split into 3 sub-chunks of 1024/512/512 steps per
  partition so each sub-chunk's tree overlaps the next one's stream build
  and only the last (small) tree is exposed at the end.  Each core ships
  its 3*128 sub-chunk matrices + per-partition gold; the host does the
  O(cores*P) ordered log-semiring combine (vectorized numpy).

* All inputs ship as one fp16 blob [par | p | w | msel | em0 | em1] in 3
  DMAs so the class masks start immediately.
"""

from contextlib import ExitStack

import numpy as np

import concourse.bass as bass
import concourse.bacc as bacc
import concourse.mybir as mybir
import concourse.tile as tile
from concourse import bass_utils

dt = mybir.dt
ALU = mybir.AluOpType
AF = mybir.ActivationFunctionType
AX = mybir.AxisListType

T = 2097152
NCORES = 8
P = 128                  # SBUF partitions
L = T // NCORES          # steps per core = 262144
F = L // P               # steps per partition = 2048
SC_SIZES = (1024, 512, 512)
SC = len(SC_SIZES)
NPOS = 19                # position classes with nonzero matrices
BSH = 8.0                # ACT positivity shift
ACT_LO = 9               # position classes >= this use ACT-produced mv
EW = 4 * SC + 1          # out row: SC matrices (4 entries each) + gold

# param row layout (f32 words): [pos' 19*4 | D 4 | VB 19*4 | negVB 19*4]
# pos' = pos + w2w[1] (folded);  D = w2w[0] - w2w[1]
NPAR = 19 * 4 + 4 + 19 * 4 + 19 * 4
COL_POS = 0
COL_W = 76
COL_VB = 80
COL_NVB = 156

W0 = 2 * NPAR + 5 * F    # fp16 blob columns


_NC_CACHE = None
LAST_RESULTS = None  # BassKernelResults of the most recent kernel() call


def _comp(i, j):
    return i * 2 + j


def _build_nc():
    nc = bacc.Bacc()

    b0_d = nc.dram_tensor("blob0", [P, W0], dt.float16, kind="ExternalInput")
    out_d = nc.dram_tensor("out", [P, EW], dt.float32, kind="ExternalOutput")

    with ExitStack() as ctx:
        tc = ctx.enter_context(tile.TileContext(nc))
        pool = ctx.enter_context(tc.tile_pool(name="main", bufs=1))

        # ---- loads: [par | p | w | msel | em0 | em1] in 3 DMAs ----
        b0 = pool.tile([P, W0], dt.float16, tag="b0", name="b0")
        hq = 2 * NPAR + 1024       # par + p columns for sub-chunk 0
        h0 = 2 * NPAR + F          # par + p
        h1 = h0 + 2 * F            # + w + msel
        nc.sync.dma_start(b0[:, 0:hq], b0_d[:, 0:hq])
        nc.sync.dma_start(b0[:, hq:h0], b0_d[:, hq:h0])
        nc.sync.dma_start(b0[:, h0:h1], b0_d[:, h0:h1])
        nc.sync.dma_start(b0[:, h1:W0], b0_d[:, h1:W0])

        par32 = b0[:, 0:2 * NPAR].bitcast(dt.float32)
        p_t = b0[:, 2 * NPAR:h0]
        w_t = b0[:, h0:h0 + F]
        msel_t = b0[:, h0 + F:h1]
        em0_t = b0[:, h1:h1 + F]
        em1_t = b0[:, h1 + F:W0]

        def V(col):
            return par32[:, col:col + 1]

        bias_c = {}
        for c in range(ACT_LO, NPOS):
            t_ = pool.tile([P, 1], dt.float32, tag=f"bc{c}", name=f"bc{c}")
            nc.vector.memset(t_[:], -float(c))
            bias_c[c] = t_
        bconst = pool.tile([P, 1], dt.float32, tag="bconst", name="bconst")
        nc.vector.memset(bconst[:], BSH)

        FSMAX = max(SC_SIZES)
        HSMAX = FSMAX // 2
        SC_OFF = [sum(SC_SIZES[:i]) for i in range(SC)]
        acc = [
            pool.tile([P, F], dt.float16, tag=f"acc{c}", name=f"acc{c}")
            for c in range(4)
        ]
        # mv work tiles, full-F, sliced per sub-chunk (cross-SC skew spaces
        # same-SC reuse)
        mv = [
            pool.tile([P, F], dt.float16, tag=f"mv{i}", name=f"mv{i}")
            for i in range(4 * 4)
        ]

        def mv_t(ki, c, s):
            return mv[ki * 4 + c][:, SC_OFF[s]:SC_OFF[s] + SC_SIZES[s]]

        amv = [
            pool.tile([P, F], dt.float16, tag=f"amv{i}", name=f"amv{i}")
            for i in range(3 * 4)
        ]

        def amv_t(ki, c, s):
            return amv[ki * 4 + c][:, SC_OFF[s]:SC_OFF[s] + SC_SIZES[s]]

        sqt = [
            pool.tile([P, F], dt.float16, tag=f"sq{i}", name=f"sq{i}")
            for i in range(2)
        ]
        mvb = pool.tile([P, FSMAX], dt.float16, tag="mvb", name="mvb")
        qm = [
            pool.tile([P, FSMAX], dt.float16, tag=f"qm{i}", name=f"qm{i}")
            for i in range(2)
        ]
        gprod = [
            pool.tile([P, FSMAX], dt.float16, tag=f"gp{i}", name=f"gp{i}")
            for i in range(2)
        ]
        gcell = pool.tile([P, 4 * SC], dt.float32, tag="gcell", name="gcell")
        res = pool.tile([P, EW], dt.float32, tag="res", name="res")

        # striped DVE/Pool assignment for accumulate adds
        POOL_NUM, POOL_DEN = 1, 3
        add_ctr = [0] * 4

        def add_eng(comp):
            add_ctr[comp] += 1
            k = (add_ctr[comp] + comp) % POOL_DEN
            return nc.gpsimd if k < POOL_NUM else nc.vector

        def tadd(comp, out, a, b):
            add_eng(comp).tensor_add(out, a, b)

        def sc_views(s):
            sl = slice(SC_OFF[s], SC_OFF[s] + SC_SIZES[s])
            return (p_t[:, sl], w_t[:, sl], msel_t[:, sl],
                    [a[:, sl] for a in acc], sl)

        dve_classes = list(range(0, ACT_LO))
        dgroups = [dve_classes[i:i + 4]
                   for i in range(0, len(dve_classes), 4)]
        act_classes = list(range(ACT_LO, NPOS))
        agroups = [act_classes[i:i + 3]
                   for i in range(0, len(act_classes), 3)]

        def emit_dve_group(s, gi):
            grp = dgroups[gi]
            p_s, w_s, m_s, acc_s, sl = sc_views(s)
            FS = SC_SIZES[s]
            for c in range(4):
                tiles = []
                for ki, k in enumerate(grp):
                    m = mv_t(ki, c, s)
                    nc.vector.tensor_scalar(
                        m, p_s, float(k), V(COL_POS + 4 * k + c),
                        ALU.is_equal, ALU.mult,
                    )
                    tiles.append(m)
                if len(tiles) == 4:
                    tadd(c, tiles[0], tiles[0], tiles[1])
                    tadd(c, tiles[2], tiles[2], tiles[3])
                    if gi == 0:
                        tadd(c, acc_s[c], tiles[0], tiles[2])
                    else:
                        tadd(c, tiles[0], tiles[0], tiles[2])
                        tadd(c, acc_s[c], acc_s[c], tiles[0])
                else:
                    while len(tiles) > 1:
                        tadd(c, tiles[0], tiles[0], tiles[1])
                        tiles = [tiles[0]] + tiles[2:]
                    if gi == 0:
                        nc.vector.tensor_copy(acc_s[c], tiles[0])
                    else:
                        tadd(c, acc_s[c], acc_s[c], tiles[0])

        def emit_act_group(s, gi):
            grp = agroups[gi]
            p_s, w_s, m_s, acc_s, sl = sc_views(s)
            FS = SC_SIZES[s]
            for ki, k in enumerate(grp):
                sq = sqt[ki % 2][:, SC_OFF[s]:SC_OFF[s] + FS]
                nc.scalar.activation(sq, p_s, AF.Square, bias=bias_c[k][:])
                for c in range(4):
                    nc.scalar.activation(
                        amv_t(ki, c, s), sq, AF.Relu,
                        bias=V(COL_VB + 4 * k + c),
                        scale=V(COL_NVB + 4 * k + c),
                    )
            for c in range(4):
                tiles = [amv_t(ki, c, s) for ki in range(len(grp))]
                while len(tiles) > 1:
                    tadd(c, tiles[0], tiles[0], tiles[1])
                    tiles = [tiles[0]] + tiles[2:]
                tadd(c, acc_s[c], acc_s[c], tiles[0])

        def emit_sub_w_em(s):
            p_s, w_s, m_s, acc_s, sl = sc_views(s)
            FS = SC_SIZES[s]
            # remove the spurious +B over [p >= ACT_LO] (includes the t=0
            # pad step p=19; host adds B back to both outputs)
            nc.vector.tensor_scalar(
                mvb[:, 0:FS], p_s, ACT_LO - 0.5, bconst[:], ALU.is_gt,
                ALU.mult,
            )
            for c in range(4):
                add_eng(c).tensor_sub(acc_s[c], acc_s[c], mvb[:, 0:FS])
            # who2who: w2w[1] is folded into the position table, so only
            # (w==0)*(w2w[0]-w2w[1]) remains (w==2 occurs only at t=0)
            for c in range(4):
                m0 = mv_t(0, c, s)
                nc.vector.tensor_scalar(
                    m0, w_s, 0.0, V(COL_W + c), ALU.is_equal, ALU.mult,
                )
                tadd(c, acc_s[c], acc_s[c], m0)
            # emission fold: acc[i,j] += em_j
            for c in range(4):
                em_s = (em0_t if c % 2 == 0 else em1_t)[:, sl]
                tadd(c, acc_s[c], acc_s[c], em_s)

        def emit_gold(s):
            p_s, w_s, m_s, acc_s, sl = sc_views(s)
            FS = SC_SIZES[s]
            for pair in range(4):
                q = qm[pair % 2][:, 0:FS]
                nc.vector.tensor_scalar(q, m_s, float(pair), None,
                                        ALU.is_equal)
                pr = gprod[pair % 2][:, 0:FS]
                nc.vector.tensor_mul(pr, q, acc_s[pair])
                nc.vector.tensor_scalar(
                    pr, pr, 1.0, None, ALU.mult, ALU.add,
                    accum_out=gcell[:, s * 4 + pair:s * 4 + pair + 1],
                )

        def u2(ap):
            return ap.unsqueeze(2).unsqueeze(3)

        def emit_tree(s):
            p_s, w_s, m_s, acc_s, sl = sc_views(s)
            FS = SC_SIZES[s]
            HS = FS // 2
            FP16_LEVELS = 4
            X16 = pool.tile([P, HSMAX, 2, 2], dt.float16, tag="X16",
                            name="X16")[:, 0:HS]
            Y16a = pool.tile([P, HSMAX, 2, 2], dt.float16, tag="Y16a",
                             name="Y16a")[:, 0:HS]
            Y16b = pool.tile([P, HSMAX // 2, 2, 2], dt.float16, tag="Y16b",
                             name="Y16b")[:, 0:HS // 2]
            X32 = pool.tile([P, HSMAX // 16, 2, 2], dt.float32, tag="X32",
                            name="X32")[:, 0:max(HS // 16, 1)]
            Y0 = pool.tile([P, HSMAX, 2, 2], dt.float32, tag="Y0",
                           name="Y0")[:, 0:HS]
            Y1 = pool.tile([P, HSMAX // 2, 2, 2], dt.float32, tag="Y1",
                           name="Y1")[:, 0:HS // 2]
            for i in range(2):
                for j in range(2):
                    add_eng(_comp(i, j)).tensor_add(
                        X16[:, :, i:i + 1, j:j + 1],
                        u2(acc_s[_comp(i, 0)][:, 0::2]),
                        u2(acc_s[_comp(0, j)][:, 1::2]),
                    )
                    add_eng(_comp(i, j)).tensor_add(
                        Y16a[:, :, i:i + 1, j:j + 1],
                        u2(acc_s[_comp(i, 1)][:, 0::2]),
                        u2(acc_s[_comp(1, j)][:, 1::2]),
                    )
            nc.vector.tensor_sub(Y16a[:], Y16a[:], X16[:])
            nc.scalar.activation(Y0[:], Y16a[:], AF.Exp)
            nc.scalar.activation(Y0[:], Y0[:], AF.Ln, bias=1.0)
            mlev = pool.tile([P, HSMAX, 2, 2], dt.float16, tag="m1",
                             name="m1")[:, 0:HS]
            nc.vector.tensor_add(mlev[:], X16[:], Y0[:])

            w_cur = HS
            lev = 1
            while w_cur > 1:
                w2 = w_cur // 2
                lev += 1
                sh = [P, w2, 2, 2]
                a_i0 = mlev[:, 0:w_cur:2, :, 0:1].broadcast_to(sh)
                a_i1 = mlev[:, 0:w_cur:2, :, 1:2].broadcast_to(sh)
                b_0j = mlev[:, 1:w_cur:2, 0:1, :].broadcast_to(sh)
                b_1j = mlev[:, 1:w_cur:2, 1:2, :].broadcast_to(sh)
                sp = (Y0 if lev % 2 == 1 else Y1)[:, 0:w2]
                if lev <= FP16_LEVELS:
                    xv = X16[:, 0:w2]
                    yv = (Y16a if lev % 2 == 1 else Y16b)[:, 0:w2]
                    (add_eng(lev % 4) if w2 >= 64 else nc.vector).tensor_add(
                        xv, a_i0, b_0j)
                    (add_eng((lev + 2) % 4) if w2 >= 64 else nc.vector
                     ).tensor_add(yv, a_i1, b_1j)
                    nc.vector.tensor_sub(yv, yv, xv)
                    nc.scalar.activation(sp, yv, AF.Exp)
                else:
                    xv = X32[:, 0:w2]
                    yv = sp
                    nc.vector.tensor_add(xv, a_i0, b_0j)
                    nc.vector.tensor_add(yv, a_i1, b_1j)
                    nc.vector.tensor_sub(yv, yv, xv)
                    nc.scalar.activation(sp, sp, AF.Exp)
                nc.scalar.activation(sp, sp, AF.Ln, bias=1.0)
                mdt = dt.float16 if lev <= FP16_LEVELS else dt.float32
                mwidth = max(HSMAX // (2 ** (lev - 1)), 1)
                mnext = pool.tile([P, mwidth, 2, 2], mdt, tag=f"m{lev}",
                                  name=f"m{lev}")[:, 0:w2]
                nc.vector.tensor_add(mnext[:], xv, sp)
                mlev = mnext
                w_cur = w2
            nc.vector.tensor_copy(
                res[:, 4 * s:4 * s + 4].rearrange("p (a b c) -> p a b c",
                                                  a=1, b=2),
                mlev[:],
            )

        # ---- skewed block emission: sub-chunk s trails s-1 by SKEW blocks
        # so completions stagger and each tree overlaps the next build ----
        SKEW = 4
        blocks = []
        for s in range(SC):
            seq = []
            seq.append(lambda s=s: emit_dve_group(s, 0))
            for gi in range(len(agroups)):
                seq.append(lambda s=s, gi=gi: emit_act_group(s, gi))
                if gi + 1 < len(dgroups):
                    seq.append(lambda s=s, gi=gi: emit_dve_group(s, gi + 1))
            seq.append(lambda s=s: emit_sub_w_em(s))
            seq.append(lambda s=s: emit_gold(s))
            seq.append(lambda s=s: emit_tree(s))
            for bi, fn in enumerate(seq):
                blocks.append((bi + SKEW * s, s, fn))
        blocks.sort(key=lambda kv: (kv[0], kv[1]))
        for _, _, fn in blocks:
            fn()

        # ---- gold column and store; host combines ----
        nc.vector.reduce_sum(res[:, 4 * SC:4 * SC + 1], gcell[:], axis=AX.X)
        nc.sync.dma_start(out_d[:], res[:])

    nc.compile()

    # Exp/Ln/Square/Relu all live in 'natural_log_exp_and_others', but
    # insert_act_table_loads picks the first set containing each function,
    # emitting alternating table reloads (1.3 us each).  Retarget every load
    # to the combined set and drop the now-redundant ones.
    from concourse.hw_specs import get_activation_tables

    tables = list(get_activation_tables(nc.m.arch).keys())
    combined = tables.index("natural_log_exp_and_others")
    for b in nc.bb_map.values():
        insts = b.bb.instructions
        kept = []
        seen_load = False
        for ins in insts:
            if ins.opcode == "LoadActFuncSet":
                si = ins.sync_info
                assert not (si and (si.on_wait or si.on_update)), ins.name
                if seen_load:
                    continue
                ins.act_func_set_id = combined
                seen_load = True
            kept.append(ins)
        if len(kept) != len(insts):
            b.bb.instructions = kept
    return nc


def _get_nc():
    global _NC_CACHE
    if _NC_CACHE is None:
        _NC_CACHE = _build_nc()
    return _NC_CACHE


def _f16_candidates(x, grid_pow=None):
    """Nearest fp16 (or 2^grid_pow-grid) value and its other-side neighbor."""
    if grid_pow is None:
        lo = np.float16(x)
        res = float(x) - float(lo)
        if res == 0.0:
            return np.float32(lo), np.float32(lo)
        hi = np.nextafter(lo, np.float16(np.inf if res > 0 else -np.inf),
                          dtype=np.float16)
        return np.float32(lo), np.float32(hi)
    g = 2.0 ** grid_pow
    lo = np.floor(float(x) / g) * g
    hi = lo + g
    if abs(float(x) - lo) <= abs(hi - float(x)):
        return np.float32(lo), np.float32(hi)
    return np.float32(hi), np.float32(lo)


def _optimize_tables(pos, w2w):
    """fp16 tables with per-entry rounding direction chosen so the
    systematic selected-sum bias over the reachable (p,w) cells cancels.
    w2w[1] is folded into the position table (P' = pos + w2w[1]); the
    residual class value is D = w2w[0] - w2w[1].  ACT-suffix rows sit on
    the 2^-7 grid so V+BSH stays fp16-exact."""
    posr = pos.reshape(NPOS, 4).astype(np.float64)
    wr = w2w.reshape(2, 4).astype(np.float64)
    Pp = posr + wr[1]
    Dv = wr[0] - wr[1]

    Pc = np.zeros((NPOS, 4, 2), np.float32)
    for k in range(NPOS):
        gp = -7 if k >= ACT_LO else None
        for c in range(4):
            Pc[k, c] = _f16_candidates(Pp[k, c], gp)
    Dc = np.zeros((4, 2), np.float32)
    for c in range(4):
        Dc[c] = _f16_candidates(Dv[c])

    # cell (k, w=1): value = P16[k];  cell (k, w=0): fp16(P16[k] + D16)
    e1 = posr + wr[1]
    e0 = posr + wr[0]
    d1 = Pc.astype(np.float64) - e1[:, :, None]               # [19,4,2]
    d0 = ((Pc[:, :, :, None].astype(np.float16)
           + Dc[None, :, None, :].astype(np.float16)).astype(np.float16)
          .astype(np.float64) - e0[:, :, None, None])          # [19,4,2,2]

    Ps = np.zeros((NPOS, 4), np.intp)
    Ds = np.zeros(4, np.intp)

    def total():
        s = 0.0
        for k in range(NPOS):
            for c in range(4):
                s += d1[k, c, Ps[k, c]] + d0[k, c, Ps[k, c], Ds[c]]
        return s

    best = total()
    for _ in range(4):
        improved = False
        for k in range(NPOS):
            for c in range(4):
                Ps[k, c] ^= 1
                t2 = total()
                if abs(t2) < abs(best):
                    best = t2
                    improved = True
                else:
                    Ps[k, c] ^= 1
        for c in range(4):
            Ds[c] ^= 1
            t2 = total()
            if abs(t2) < abs(best):
                best = t2
                improved = True
            else:
                Ds[c] ^= 1
        if not improved:
            break

    P16 = np.take_along_axis(Pc, Ps[:, :, None], axis=2)[:, :, 0]
    D16 = Dc[np.arange(4), Ds]
    return P16.astype(np.float32), D16.astype(np.float32)


def _lse_combine(A, B):
    """ordered log-semiring 2x2 product, vectorized over leading dims"""
    return np.logaddexp(A[..., :, 0:1] + B[..., 0:1, :],
                        A[..., :, 1:2] + B[..., 1:2, :])


def kernel(**inputs):
    em = np.asarray(inputs["emission_scores"], dtype=np.float32)
    lab = np.asarray(inputs["label"]).astype(np.float32)
    w = np.asarray(inputs["who2who_state"]).astype(np.float32)
    p = np.asarray(inputs["position_state"]).astype(np.float32)
    w2w = np.asarray(inputs["who2who_params"], dtype=np.float32)
    pos = np.asarray(inputs["position_params"], dtype=np.float32)
    assert em.shape == (T, 2), em.shape

    labp = np.empty_like(lab)
    labp[0] = 0.0
    labp[1:] = lab[:-1]
    msel = (2.0 * labp + lab).astype(np.float16)

    P16, D16 = _optimize_tables(pos, w2w)
    vb = P16 + np.float32(BSH)   # fp16-exact for the ACT rows (2^-7 grid)
    par_row = np.concatenate([
        P16.reshape(-1), D16.reshape(-1), vb.reshape(-1), (-vb).reshape(-1)
    ]).astype(np.float32)
    assert par_row.shape[0] == NPAR
    par16 = np.broadcast_to(par_row.view(np.float16), (P, 2 * NPAR))

    p16 = p.astype(np.float16)
    w16 = w.astype(np.float16)
    em16 = em.astype(np.float16)

    in_maps = []
    for k in range(NCORES):
        sl = slice(k * L, (k + 1) * L)
        blob0 = np.concatenate(
            [
                par16,
                p16[sl].reshape(P, F),
                w16[sl].reshape(P, F),
                msel[sl].reshape(P, F),
                np.ascontiguousarray(em16[sl, 0].reshape(P, F)),
                np.ascontiguousarray(em16[sl, 1].reshape(P, F)),
            ],
            axis=1,
        )
        in_maps.append({"blob0": np.ascontiguousarray(blob0)})

    nc = _get_nc()
    kr = bass_utils.run_bass_kernel_spmd(nc, in_maps, core_ids=list(range(NCORES)))
    global LAST_RESULTS
    LAST_RESULTS = kr
    results = kr.results

    # host combine: ordered product of NCORES*P*SC 2x2 matrices + gold sum
    rows = np.stack([np.asarray(r["out"], dtype=np.float64) for r in results])
    gold = rows[:, :, 4 * SC].sum()
    mats = rows[:, :, 0:4 * SC].reshape(NCORES * P * SC, 2, 2)
    # pairwise tree keeps it fast and stable
    while mats.shape[0] > 1:
        n = mats.shape[0]
        even = mats[0:n - 1:2]
        odd = mats[1:n:2]
        comb = _lse_combine(even, odd)
        if n % 2 == 1:
            comb = np.concatenate([comb, mats[n - 1:n]], axis=0)
        mats = comb
    total = np.logaddexp.reduce(mats.reshape(-1))
    # the single t=0 pad step (p=19) carries the -BSH shift: add it back
    gold += BSH
    total += BSH
    return np.stack([gold, total]).astype(np.float32)


if __name__ == "__main__":
    rng = np.random.default_rng(0)
    demo = dict(
        emission_scores=rng.standard_normal((T, 2)).astype(np.float32),
        label=rng.integers(0, 2, T),
        who2who_state=np.concatenate([[2], rng.integers(0, 2, T - 1)]),
        position_state=np.concatenate([[19], rng.integers(0, 19, T - 1)]),
        who2who_params=rng.standard_normal((2, 2, 2)).astype(np.float32),
        position_params=rng.standard_normal((19, 2, 2)).astype(np.float32),
    )
    print(kernel(**demo))


# revision 17
# speedup vs baseline: 1.3411x; 1.0934x over previous
"""Trainium2 Bass kernel for a 2-state linear-chain CRF loss (BiLSTM-CRF loss_fn).

Computes, for a single conversation of length T = 2,097,152:
  gold_score  = sum_t em[t, lab[t]] + sum_{t>0} trans[t][lab[t-1], lab[t]]
  total_score = logsumexp of the CRF forward recursion
where trans[t] = who2who_sub[w[t]] + position_sub[p[t]] (60 possible 2x2
matrices; indices 2/19 select an all-zero padding matrix).

Design (one NeuronCore per contiguous chunk of 262,144 steps, 8 cores):

* Per-step matrices: trans+em is built as 4 fp16 streams by per-class masked
  accumulation (19 position classes + 2 who2who classes + emission fold).
  Class supports are disjoint, so sums of masked values are exact in fp16;
  the masked values are combined PAIRWISE (a small in-group tree) so the
  per-stream dependency depth is ~8 instead of 21 serial adds.  Work is
  split three ways: DVE runs fused (idx==c)*V tensor_scalars (4x fp16 mode)
  plus most adds; the ACT engine produces masked values for a suffix of
  position classes as Relu((V+B) - (V+B)*(p-c)^2) with B=4 making the peak
  positive (the spurious +B*[p>=a] is removed by one (p>a-.5)*B mask and 4
  subtracts; the t=0 pad step ends shifted by exactly -B, corrected on the
  host); GPSIMD takes a striped share of the adds.

* Gold score: the label-pair stream msel = 2*lab[t-1]+lab[t] selects one of
  the 4 finished streams per step; gold = sum_t acc[msel_t][t] via 4
  is_equal masks + multiply + accum_out per sub-chunk.  Exactness: stream
  values are single-fp16-rounded table values (+ exact-in-fp16 shifts), and
  the host chooses each table entry's fp16 rounding DIRECTION (greedy sign
  optimization over the 19x2x4 reachable cells) so the systematic selection
  bias cancels to ~1e-5 relative.

* Forward pass: the recursion is a product of 2x2 matrices in the (log, +)
  semiring; each core tree-reduces with LSE(a,b) = a + ln(1+exp(b-a)) on
  ACT.  The chunk is split into 3 sub-chunks of 1024/512/512 steps per
  partition so each sub-chunk's tree overlaps the next one's stream build
  and only the last (small) tree is exposed at the end.  Each core ships
  its 3*128 sub-chunk matrices + per-partition gold; the host does the
  O(cores*P) ordered log-semiring combine (vectorized numpy).

* All inputs ship as one fp16 blob [par | p | w | msel | em0 | em1] in 3
  DMAs so the class masks start immediately.
"""

from contextlib import ExitStack

import numpy as np

import concourse.bass as bass
import concourse.bacc as bacc
import concourse.mybir as mybir
import concourse.tile as tile
from concourse import bass_utils

dt = mybir.dt
ALU = mybir.AluOpType
AF = mybir.ActivationFunctionType
AX = mybir.AxisListType

T = 2097152
NCORES = 8
P = 128                  # SBUF partitions
L = T // NCORES          # steps per core = 262144
F = L // P               # steps per partition = 2048
SC_SIZES = (1024, 512, 512)
SC = len(SC_SIZES)
NPOS = 19                # position classes with nonzero matrices
BSH = 8.0                # ACT positivity shift
ACT_LO = 9               # position classes >= this use ACT-produced mv
EW = 4 * SC + 1          # out row: SC matrices (4 entries each) + gold

# param row layout (f32 words): [pos' 19*4 | D 4 | VB 19*4 | negVB 19*4]
# pos' = pos + w2w[1] (folded);  D = w2w[0] - w2w[1]
NPAR = 19 * 4 + 4 + 19 * 4 + 19 * 4
COL_POS = 0
COL_W = 76
COL_VB = 80
COL_NVB = 156

W0 = 2 * NPAR + 5 * F    # fp16 blob columns


_NC_CACHE = None
LAST_RESULTS = None  # BassKernelResults of the most recent kernel() call


def _comp(i, j):
    return i * 2 + j


def _build_nc():
    nc = bacc.Bacc()

    b0_d = nc.dram_tensor("blob0", [P, W0], dt.float16, kind="ExternalInput")
    out_d = nc.dram_tensor("out", [P, EW], dt.float32, kind="ExternalOutput")

    with ExitStack() as ctx:
        tc = ctx.enter_context(tile.TileContext(nc))
        pool = ctx.enter_context(tc.tile_pool(name="main", bufs=1))

        # ---- loads: [par | p | w | msel | em0 | em1] in 3 DMAs ----
        b0 = pool.tile([P, W0], dt.float16, tag="b0", name="b0")
        hq = 2 * NPAR + 1024       # par + p columns for sub-chunk 0
        h0 = 2 * NPAR + F          # par + p
        h1 = h0 + 2 * F            # + w + msel
        nc.sync.dma_start(b0[:, 0:hq], b0_d[:, 0:hq])
        nc.sync.dma_start(b0[:, hq:h0], b0_d[:, hq:h0])
        nc.sync.dma_start(b0[:, h0:h1], b0_d[:, h0:h1])
        nc.sync.dma_start(b0[:, h1:W0], b0_d[:, h1:W0])

        par32 = b0[:, 0:2 * NPAR].bitcast(dt.float32)
        p_t = b0[:, 2 * NPAR:h0]
        w_t = b0[:, h0:h0 + F]
        msel_t = b0[:, h0 + F:h1]
        em0_t = b0[:, h1:h1 + F]
        em1_t = b0[:, h1 + F:W0]

        def V(col):
            return par32[:, col:col + 1]

        bias_c = {}
        for c in range(ACT_LO, NPOS):
            t_ = pool.tile([P, 1], dt.float32, tag=f"bc{c}", name=f"bc{c}")
            nc.vector.memset(t_[:], -float(c))
            bias_c[c] = t_
        bconst = pool.tile([P, 1], dt.float32, tag="bconst", name="bconst")
        nc.vector.memset(bconst[:], BSH)

        FSMAX = max(SC_SIZES)
        HSMAX = FSMAX // 2
        SC_OFF = [sum(SC_SIZES[:i]) for i in range(SC)]
        acc = [
            pool.tile([P, F], dt.float16, tag=f"acc{c}", name=f"acc{c}")
            for c in range(4)
        ]
        # mv work tiles, full-F, sliced per sub-chunk (cross-SC skew spaces
        # same-SC reuse)
        mv = [
            pool.tile([P, F], dt.float16, tag=f"mv{i}", name=f"mv{i}")
            for i in range(4 * 4)
        ]

        def mv_t(ki, c, s):
            return mv[ki * 4 + c][:, SC_OFF[s]:SC_OFF[s] + SC_SIZES[s]]

        amv = [
            pool.tile([P, F], dt.float16, tag=f"amv{i}", name=f"amv{i}")
            for i in range(3 * 4)
        ]

        def amv_t(ki, c, s):
            return amv[ki * 4 + c][:, SC_OFF[s]:SC_OFF[s] + SC_SIZES[s]]

        sqt = [
            pool.tile([P, F], dt.float16, tag=f"sq{i}", name=f"sq{i}")
            for i in range(2)
        ]
        mvb = pool.tile([P, FSMAX], dt.float16, tag="mvb", name="mvb")
        qm = [
            pool.tile([P, FSMAX], dt.float16, tag=f"qm{i}", name=f"qm{i}")
            for i in range(2)
        ]
        gprod = [
            pool.tile([P, FSMAX], dt.float16, tag=f"gp{i}", name=f"gp{i}")
            for i in range(2)
        ]
        gcell = pool.tile([P, 4 * SC], dt.float32, tag="gcell", name="gcell")
        res = pool.tile([P, EW], dt.float32, tag="res", name="res")

        # striped DVE/Pool assignment for accumulate adds
        POOL_NUM, POOL_DEN = 1, 3
        add_ctr = [0] * 4

        def add_eng(comp):
            add_ctr[comp] += 1
            k = (add_ctr[comp] + comp) % POOL_DEN
            return nc.gpsimd if k < POOL_NUM else nc.vector

        def tadd(comp, out, a, b):
            add_eng(comp).tensor_add(out, a, b)

        def sc_views(s):
            sl = slice(SC_OFF[s], SC_OFF[s] + SC_SIZES[s])
            return (p_t[:, sl], w_t[:, sl], msel_t[:, sl],
                    [a[:, sl] for a in acc], sl)

        dve_classes = list(range(0, ACT_LO))
        dgroups = [dve_classes[i:i + 4]
                   for i in range(0, len(dve_classes), 4)]
        act_classes = list(range(ACT_LO, NPOS))
        agroups = [act_classes[i:i + 3]
                   for i in range(0, len(act_classes), 3)]

        def emit_dve_group(s, gi):
            grp = dgroups[gi]
            p_s, w_s, m_s, acc_s, sl = sc_views(s)
            FS = SC_SIZES[s]
            for c in range(4):
                tiles = []
                for ki, k in enumerate(grp):
                    m = mv_t(ki, c, s)
                    nc.vector.tensor_scalar(
                        m, p_s, float(k), V(COL_POS + 4 * k + c),
                        ALU.is_equal, ALU.mult,
                    )
                    tiles.append(m)
                if len(tiles) == 4:
                    tadd(c, tiles[0], tiles[0], tiles[1])
                    tadd(c, tiles[2], tiles[2], tiles[3])
                    if gi == 0:
                        tadd(c, acc_s[c], tiles[0], tiles[2])
                    else:
                        tadd(c, tiles[0], tiles[0], tiles[2])
                        tadd(c, acc_s[c], acc_s[c], tiles[0])
                else:
                    while len(tiles) > 1:
                        tadd(c, tiles[0], tiles[0], tiles[1])
                        tiles = [tiles[0]] + tiles[2:]
                    if gi == 0:
                        nc.vector.tensor_copy(acc_s[c], tiles[0])
                    else:
                        tadd(c, acc_s[c], acc_s[c], tiles[0])

        def emit_act_group(s, gi):
            grp = agroups[gi]
            p_s, w_s, m_s, acc_s, sl = sc_views(s)
            FS = SC_SIZES[s]
            for ki, k in enumerate(grp):
                sq = sqt[ki % 2][:, SC_OFF[s]:SC_OFF[s] + FS]
                nc.scalar.activation(sq, p_s, AF.Square, bias=bias_c[k][:])
                for c in range(4):
                    nc.scalar.activation(
                        amv_t(ki, c, s), sq, AF.Relu,
                        bias=V(COL_VB + 4 * k + c),
                        scale=V(COL_NVB + 4 * k + c),
                    )
            for c in range(4):
                tiles = [amv_t(ki, c, s) for ki in range(len(grp))]
                while len(tiles) > 1:
                    tadd(c, tiles[0], tiles[0], tiles[1])
                    tiles = [tiles[0]] + tiles[2:]
                tadd(c, acc_s[c], acc_s[c], tiles[0])

        def emit_sub_w_em(s):
            p_s, w_s, m_s, acc_s, sl = sc_views(s)
            FS = SC_SIZES[s]
            # remove the spurious +B over [p >= ACT_LO] (includes the t=0
            # pad step p=19; host adds B back to both outputs)
            nc.vector.tensor_scalar(
                mvb[:, 0:FS], p_s, ACT_LO - 0.5, bconst[:], ALU.is_gt,
                ALU.mult,
            )
            for c in range(4):
                add_eng(c).tensor_sub(acc_s[c], acc_s[c], mvb[:, 0:FS])
            # who2who: w2w[1] is folded into the position table, so only
            # (w==0)*(w2w[0]-w2w[1]) remains (w==2 occurs only at t=0)
            for c in range(4):
                m0 = mv_t(0, c, s)
                nc.vector.tensor_scalar(
                    m0, w_s, 0.0, V(COL_W + c), ALU.is_equal, ALU.mult,
                )
                tadd(c, acc_s[c], acc_s[c], m0)
            # emission fold: acc[i,j] += em_j
            for c in range(4):
                em_s = (em0_t if c % 2 == 0 else em1_t)[:, sl]
                tadd(c, acc_s[c], acc_s[c], em_s)

        def emit_gold(s):
            p_s, w_s, m_s, acc_s, sl = sc_views(s)
            FS = SC_SIZES[s]
            for pair in range(4):
                q = qm[pair % 2][:, 0:FS]
                nc.vector.tensor_scalar(q, m_s, float(pair), None,
                                        ALU.is_equal)
                pr = gprod[pair % 2][:, 0:FS]
                nc.vector.tensor_mul(pr, q, acc_s[pair])
                nc.vector.tensor_scalar(
                    pr, pr, 1.0, None, ALU.mult, ALU.add,
                    accum_out=gcell[:, s * 4 + pair:s * 4 + pair + 1],
                )

        def u2(ap):
            return ap.unsqueeze(2).unsqueeze(3)

        def emit_tree(s):
            p_s, w_s, m_s, acc_s, sl = sc_views(s)
            FS = SC_SIZES[s]
            HS = FS // 2
            FP16_LEVELS = 4
            X16 = pool.tile([P, HSMAX, 2, 2], dt.float16, tag="X16",
                            name="X16")[:, 0:HS]
            Y16a = pool.tile([P, HSMAX, 2, 2], dt.float16, tag="Y16a",
                             name="Y16a")[:, 0:HS]
            Y16b = pool.tile([P, HSMAX // 2, 2, 2], dt.float16, tag="Y16b",
                             name="Y16b")[:, 0:HS // 2]
            X32 = pool.tile([P, HSMAX // 16, 2, 2], dt.float32, tag="X32",
                            name="X32")[:, 0:max(HS // 16, 1)]
            Y0 = pool.tile([P, HSMAX, 2, 2], dt.float32, tag="Y0",
                           name="Y0")[:, 0:HS]
            Y1 = pool.tile([P, HSMAX // 2, 2, 2], dt.float32, tag="Y1",
                           name="Y1")[:, 0:HS // 2]
            for i in range(2):
                for j in range(2):
                    add_eng(_comp(i, j)).tensor_add(
                        X16[:, :, i:i + 1, j:j + 1],
                        u2(acc_s[_comp(i, 0)][:, 0::2]),
                        u2(acc_s[_comp(0, j)][:, 1::2]),
                    )
                    add_eng(_comp(i, j)).tensor_add(
                        Y16a[:, :, i:i + 1, j:j + 1],
                        u2(acc_s[_comp(i, 1)][:, 0::2]),
                        u2(acc_s[_comp(1, j)][:, 1::2]),
                    )
            nc.vector.tensor_sub(Y16a[:], Y16a[:], X16[:])
            nc.scalar.activation(Y0[:], Y16a[:], AF.Exp)
            nc.scalar.activation(Y0[:], Y0[:], AF.Ln, bias=1.0)
            mlev = pool.tile([P, HSMAX, 2, 2], dt.float16, tag="m1",
                             name="m1")[:, 0:HS]
            nc.vector.tensor_add(mlev[:], X16[:], Y0[:])

            w_cur = HS
            lev = 1
            while w_cur > 1:
                w2 = w_cur // 2
                lev += 1
                sh = [P, w2, 2, 2]
                a_i0 = mlev[:, 0:w_cur:2, :, 0:1].broadcast_to(sh)
                a_i1 = mlev[:, 0:w_cur:2, :, 1:2].broadcast_to(sh)
                b_0j = mlev[:, 1:w_cur:2, 0:1, :].broadcast_to(sh)
                b_1j = mlev[:, 1:w_cur:2, 1:2, :].broadcast_to(sh)
                sp = (Y0 if lev % 2 == 1 else Y1)[:, 0:w2]
                if lev <= FP16_LEVELS:
                    xv = X16[:, 0:w2]
                    yv = (Y16a if lev % 2 == 1 else Y16b)[:, 0:w2]
                    (add_eng(lev % 4) if w2 >= 64 else nc.vector).tensor_add(
                        xv, a_i0, b_0j)
                    (add_eng((lev + 2) % 4) if w2 >= 64 else nc.vector
                     ).tensor_add(yv, a_i1, b_1j)
                    nc.vector.tensor_sub(yv, yv, xv)
                    nc.scalar.activation(sp, yv, AF.Exp)
                else:
                    xv = X32[:, 0:w2]
                    yv = sp
                    nc.vector.tensor_add(xv, a_i0, b_0j)
                    nc.vector.tensor_add(yv, a_i1, b_1j)
                    nc.vector.tensor_sub(yv, yv, xv)
                    nc.scalar.activation(sp, sp, AF.Exp)
                nc.scalar.activation(sp, sp, AF.Ln, bias=1.0)
                mdt = dt.float16 if lev <= FP16_LEVELS else dt.float32
                mwidth = max(HSMAX // (2 ** (lev - 1)), 1)
                mnext = pool.tile([P, mwidth, 2, 2], mdt, tag=f"m{lev}",
                                  name=f"m{lev}")[:, 0:w2]
                nc.vector.tensor_add(mnext[:], xv, sp)
                mlev = mnext
                w_cur = w2
            nc.vector.tensor_copy(
                res[:, 4 * s:4 * s + 4].rearrange("p (a b c) -> p a b c",
                                                  a=1, b=2),
                mlev[:],
            )

        # ---- skewed block emission: sub-chunk s trails s-1 by SKEW blocks
        # so completions stagger and each tree overlaps the next build ----
        SKEW = 4
        blocks = []
        for s in range(SC):
            seq = []
            seq.append(lambda s=s: emit_dve_group(s, 0))
            for gi in range(len(agroups)):
                seq.append(lambda s=s, gi=gi: emit_act_group(s, gi))
                if gi + 1 < len(dgroups):
                    seq.append(lambda s=s, gi=gi: emit_dve_group(s, gi + 1))
            seq.append(lambda s=s: emit_sub_w_em(s))
            seq.append(lambda s=s: emit_gold(s))
            seq.append(lambda s=s: emit_tree(s))
            for bi, fn in enumerate(seq):
                blocks.append((bi + SKEW * s, s, fn))
        blocks.sort(key=lambda kv: (kv[0], kv[1]))
        for _, _, fn in blocks:
            fn()

        # ---- gold column and store; host combines ----
        nc.vector.reduce_sum(res[:, 4 * SC:4 * SC + 1], gcell[:], axis=AX.X)
        nc.sync.dma_start(out_d[:], res[:])

    nc.compile()

    # Exp/Ln/Square/Relu all live in 'natural_log_exp_and_others', but
    # insert_act_table_loads picks the first set containing each function,
    # emitting alternating table reloads (1.3 us each).  Retarget every load
    # to the combined set and drop the now-redundant ones.
    from concourse.hw_specs import get_activation_tables

    tables = list(get_activation_tables(nc.m.arch).keys())
    combined = tables.index("natural_log_exp_and_others")
    for b in nc.bb_map.values():
        insts = b.bb.instructions
        kept = []
        seen_load = False
        for ins in insts:
            if ins.opcode == "LoadActFuncSet":
                si = ins.sync_info
                assert not (si and (si.on_wait or si.on_update)), ins.name
                if seen_load:
                    continue
                ins.act_func_set_id = combined
                seen_load = True
            kept.append(ins)
        if len(kept) != len(insts):
            b.bb.instructions = kept
    return nc


def _get_nc():
    global _NC_CACHE
    if _NC_CACHE is None:
        _NC_CACHE = _build_nc()
    return _NC_CACHE


def _f16_candidates(x, grid_pow=None):
    """Nearest fp16 (or 2^grid_pow-grid) value and its other-side neighbor."""
    if grid_pow is None:
        lo = np.float16(x)
        res = float(x) - float(lo)
        if res == 0.0:
            return np.float32(lo), np.float32(lo)
        hi = np.nextafter(lo, np.float16(np.inf if res > 0 else -np.inf),
                          dtype=np.float16)
        return np.float32(lo), np.float32(hi)
    g = 2.0 ** grid_pow
    lo = np.floor(float(x) / g) * g
    hi = lo + g
    if abs(float(x) - lo) <= abs(hi - float(x)):
        return np.float32(lo), np.float32(hi)
    return np.float32(hi), np.float32(lo)


def _optimize_tables(pos, w2w):
    """fp16 tables with per-entry rounding direction chosen so the
    systematic selected-sum bias over the reachable (p,w) cells cancels.
    w2w[1] is folded into the position table (P' = pos + w2w[1]); the
    residual class value is D = w2w[0] - w2w[1].  ACT-suffix rows sit on
    the 2^-7 grid so V+BSH stays fp16-exact."""
    posr = pos.reshape(NPOS, 4).astype(np.float64)
    wr = w2w.reshape(2, 4).astype(np.float64)
    Pp = posr + wr[1]
    Dv = wr[0] - wr[1]

    Pc = np.zeros((NPOS, 4, 2), np.float32)
    for k in range(NPOS):
        gp = -7 if k >= ACT_LO else None
        for c in range(4):
            Pc[k, c] = _f16_candidates(Pp[k, c], gp)
    Dc = np.zeros((4, 2), np.float32)
    for c in range(4):
        Dc[c] = _f16_candidates(Dv[c])

    # cell (k, w=1): value = P16[k];  cell (k, w=0): fp16(P16[k] + D16)
    e1 = posr + wr[1]
    e0 = posr + wr[0]
    d1 = Pc.astype(np.float64) - e1[:, :, None]               # [19,4,2]
    d0 = ((Pc[:, :, :, None].astype(np.float16)
           + Dc[None, :, None, :].astype(np.float16)).astype(np.float16)
          .astype(np.float64) - e0[:, :, None, None])          # [19,4,2,2]

    Ps = np.zeros((NPOS, 4), np.intp)
    Ds = np.zeros(4, np.intp)

    def total():
        s = 0.0
        for k in range(NPOS):
            for c in range(4):
                s += d1[k, c, Ps[k, c]] + d0[k, c, Ps[k, c], Ds[c]]
        return s

    best = total()
    for _ in range(4):
        improved = False
        for k in range(NPOS):
            for c in range(4):
                Ps[k, c] ^= 1
                t2 = total()
                if abs(t2) < abs(best):
                    best = t2
                    improved = True
                else:
                    Ps[k, c] ^= 1
        for c in range(4):
            Ds[c] ^= 1
            t2 = total()
            if abs(t2) < abs(best):
                best = t2
                improved = True
            else:
                Ds[c] ^= 1
        if not improved:
            break

    P16 = np.take_along_axis(Pc, Ps[:, :, None], axis=2)[:, :, 0]
    D16 = Dc[np.arange(4), Ds]
    return P16.astype(np.float32), D16.astype(np.float32)


def _lse_combine(A, B):
    """ordered log-semiring 2x2 product, vectorized over leading dims"""
    return np.logaddexp(A[..., :, 0:1] + B[..., 0:1, :],
                        A[..., :, 1:2] + B[..., 1:2, :])


def kernel(**inputs):
    em = np.asarray(inputs["emission_scores"], dtype=np.float32)
    lab = np.asarray(inputs["label"]).astype(np.float32)
    w = np.asarray(inputs["who2who_state"]).astype(np.float32)
    p = np.asarray(inputs["position_state"]).astype(np.float32)
    w2w = np.asarray(inputs["who2who_params"], dtype=np.float32)
    pos = np.asarray(inputs["position_params"], dtype=np.float32)
    assert em.shape == (T, 2), em.shape

    labp = np.empty_like(lab)
    labp[0] = 0.0
    labp[1:] = lab[:-1]
    msel = (2.0 * labp + lab).astype(np.float16)

    P16, D16 = _optimize_tables(pos, w2w)
    vb = P16 + np.float32(BSH)   # fp16-exact for the ACT rows (2^-7 grid)
    par_row = np.concatenate([
        P16.reshape(-1), D16.reshape(-1), vb.reshape(-1), (-vb).reshape(-1)
    ]).astype(np.float32)
    assert par_row.shape[0] == NPAR
    par16 = np.broadcast_to(par_row.view(np.float16), (P, 2 * NPAR))

    p16 = p.astype(np.float16)
    w16 = w.astype(np.float16)
    em16 = em.astype(np.float16)

    in_maps = []
    for k in range(NCORES):
        sl = slice(k * L, (k + 1) * L)
        blob0 = np.concatenate(
            [
                par16,
                p16[sl].reshape(P, F),
                w16[sl].reshape(P, F),
                msel[sl].reshape(P, F),
                np.ascontiguousarray(em16[sl, 0].reshape(P, F)),
                np.ascontiguousarray(em16[sl, 1].reshape(P, F)),
            ],
            axis=1,
        )
        in_maps.append({"blob0": np.ascontiguousarray(blob0)})

    nc = _get_nc()
    kr = bass_utils.run_bass_kernel_spmd(nc, in_maps, core_ids=list(range(NCORES)))
    global LAST_RESULTS
    LAST_RESULTS = kr
    results = kr.results

    # host combine: ordered product of NCORES*P*SC 2x2 matrices + gold sum
    rows = np.stack([np.asarray(r["out"], dtype=np.float64) for r in results])
    gold = rows[:, :, 4 * SC].sum()
    mats = rows[:, :, 0:4 * SC].reshape(NCORES * P * SC, 2, 2)
    # pairwise tree keeps it fast and stable
    while mats.shape[0] > 1:
        n = mats.shape[0]
        even = mats[0:n - 1:2]
        odd = mats[1:n:2]
        comb = _lse_combine(even, odd)
        if n % 2 == 1:
            comb = np.concatenate([comb, mats[n - 1:n]], axis=0)
        mats = comb
    total = np.logaddexp.reduce(mats.reshape(-1))
    # the single t=0 pad step (p=19) carries the -BSH shift: add it back
    gold += BSH
    total += BSH
    return np.stack([gold, total]).astype(np.float32)


if __name__ == "__main__":
    rng = np.random.default_rng(0)
    demo = dict(
        emission_scores=rng.standard_normal((T, 2)).astype(np.float32),
        label=rng.integers(0, 2, T),
        who2who_state=np.concatenate([[2], rng.integers(0, 2, T - 1)]),
        position_state=np.concatenate([[19], rng.integers(0, 19, T - 1)]),
        who2who_params=rng.standard_normal((2, 2, 2)).astype(np.float32),
        position_params=rng.standard_normal((19, 2, 2)).astype(np.float32),
    )
    print(kernel(**demo))
